# revision 1
# baseline (speedup 1.0000x reference)
"""DRew-GIN layer on 8 TRN2 NeuronCores.

Strategy (source-sharded, no table replication):
  - Nodes are sharded 8 ways. Core c computes the three coef-scaled hop MLP
    tables h'_k = hop_coef[k-1] * MLP_k(emb_src_k) for its node slice only
    (plus the self-loop MLP for its slice), node-major f32 in local DRAM.
  - Edges are partitioned by SOURCE core. Each core produces a PARTIAL
    aggregate over the full (padded) destination range: edges are sorted by
    destination window (128 dest rows); per 128-edge tile we dma_gather the
    source rows from the local h' table, build a one-hot selection matrix
    S^T[e, d] = (slot[e] == d) on DVE, and matmul-accumulate S^T.T @ G into
    the window's PSUM tile. Window flushes go to a partial table
    [8*DSLICE_PAD, 128] f32.
  - One ReduceScatter (CCE add) sums the 8 partials and hands core c its own
    destination slice; add the self-term and write the output slice.

Everything is f32 end-to-end (gather rows are 512B = DMA line-rate minimum).
"""

import math
import sys

sys.path.insert(0, "/opt/trn_rl_repo")

import numpy as np

import concourse.bacc as bacc
import concourse.bass as bass
import concourse.tile as tile
from concourse import mybir
from concourse.bass_utils import run_bass_kernel_spmd

NCORES = 8
C = 128
P = 128
GBLK = 32  # gather block = 32 tiles = 4096 indices


def make_cfg(n_nodes, n_edges):
    assert n_nodes % NCORES == 0
    slice_ = n_nodes // NCORES
    slice_pad = ((slice_ + P - 1) // P) * P
    wps = slice_pad // P  # windows per dest slice
    cfg = dict(
        N=n_nodes,
        E=n_edges,
        SLICE=slice_,
        SLICE_PAD=slice_pad,
        TBL=3 * slice_pad,  # h' table rows per core
        DSLICE_PAD=slice_pad,
        WINDOWS=NCORES * wps,
        WPS=wps,
    )
    return cfg


# ---------------------------------------------------------------- host prep


def prep_edges(cfg, row, col, ew):
    """Returns (per_core {gidx,slots}, meta {tile_window, T_total, B})."""
    N, SLICE, SLICE_PAD, DSLICE_PAD, WINDOWS = (
        cfg["N"],
        cfg["SLICE"],
        cfg["SLICE_PAD"],
        cfg["DSLICE_PAD"],
        cfg["WINDOWS"],
    )
    row = row.astype(np.int64)
    col = col.astype(np.int64)
    ew = ew.astype(np.int64)
    s = col // SLICE
    local = col - s * SLICE
    trow = (ew - 1) * SLICE_PAD + local
    assert trow.max() < 3 * SLICE_PAD <= 32767
    dp = (row // SLICE) * DSLICE_PAD + (row % SLICE)
    w = dp // P
    slot = dp % P

    key = s * WINDOWS + w
    order = np.argsort(key, kind="stable")
    key_s = key[order]
    counts = np.bincount(key_s, minlength=NCORES * WINDOWS).reshape(NCORES, WINDOWS)
    tw = np.maximum(1, (counts.max(axis=0) + P - 1) // P)  # [WINDOWS]
    T_total = int(tw.sum())
    B = (T_total + GBLK - 1) // GBLK
    tile_window = np.repeat(np.arange(WINDOWS), tw)  # [T_total]
    win_tile_off = np.concatenate([[0], np.cumsum(tw)])[:-1]  # [WINDOWS]

    # position of each (sorted) edge inside its (core, window) group
    group_starts = np.concatenate([[0], np.cumsum(counts.reshape(-1))])[:-1]
    pos_in_group = np.arange(len(key_s)) - group_starts[key_s]
    # destination slot index in the padded per-core stream
    core_of = key_s // WINDOWS
    win_of = key_s % WINDOWS
    stream_pos = win_tile_off[win_of] * P + pos_in_group

    NPAD = T_total * P
    per_core = []
    trow_s = trow[order]
    slot_s = slot[order]
    for c in range(NCORES):
        m = core_of == c
        gidx = np.zeros(NPAD, np.int16)  # dummy -> row 0
        slots = np.full(NPAD, 255, np.float32)  # dummy -> no slot match
        gidx[stream_pos[m]] = trow_s[m].astype(np.int16)
        slots[stream_pos[m]] = slot_s[m].astype(np.float32)
        # wrap gidx for dma_gather: block b, idx j -> [j%16, b*256 + j//16]
        blk = np.zeros(B * GBLK * P, np.int16)
        blk[:NPAD] = gidx
        blk = blk.reshape(B, GBLK * P // 16, 16).transpose(0, 2, 1)  # [B,16,256]
        blk = np.tile(blk, (1, 8, 1))  # [B,128,256]
        gidx_in = blk.transpose(1, 0, 2).reshape(P, -1).copy()  # [128, B*256]
        slots_in = slots.reshape(T_total, P).T.copy()  # [128, T_total]
        per_core.append(dict(gidx=gidx_in, slots=slots_in))
    meta = dict(tile_window=tile_window.tolist(), T_total=T_total, B=B)
    return per_core, meta


# ---------------------------------------------------------------- builder


def build_kernel(cfg, meta, debug_phases=4, comm_mode="a2a"):
    SLICE, SLICE_PAD, TBL, DSLICE_PAD, WINDOWS = (
        cfg["SLICE"],
        cfg["SLICE_PAD"],
        cfg["TBL"],
        cfg["DSLICE_PAD"],
        cfg["WINDOWS"],
    )
    T_total, B, tile_window = meta["T_total"], meta["B"], meta["tile_window"]
    PTOT = NCORES * DSLICE_PAD
    f32 = mybir.dt.float32
    SGRP = 8          # one-hot compare batch (tiles per DVE op)
    WQ = 4            # windows per PSUM bank (quad-window flush)
    assert WINDOWS % WQ == 0

    nc = bacc.Bacc()

    def param(name, shape, dt=f32):
        return nc.declare_dram_parameter(name, list(shape), dt, isOutput=False)

    embp = [param(n, [P, SLICE_PAD]) for n in ("embA", "embB", "embC")]
    w1 = [param(f"w1_{i}", [P, P]) for i in range(4)]  # rel, hh0, hh1, loop
    b1 = [param(f"b1_{i}", [P, 1]) for i in range(4)]
    w2 = [param(f"w2_{i}", [P, P]) for i in range(4)]
    b2 = [param(f"b2_{i}", [P, P]) for i in range(4)]  # row-broadcast
    coef = param("coef", [1, 4])  # hop coefs (4th = 1.0 for self)
    iota_p = param("iota", [P, P])
    gidx_p = param("gidx", [P, B * 256], mybir.dt.int16)
    slots_p = param("slots", [P, T_total])
    tok_p = param("tok", [1, 1])
    out_ext = nc.declare_dram_parameter("out", [SLICE, C], f32, isOutput=True)
    tok_out = nc.declare_dram_parameter("tok_out", [1, 1], f32, isOutput=True)

    h_dram = nc.dram_tensor("h_tbl", [TBL, C], f32)
    self_dram = nc.dram_tensor("self_tbl", [SLICE_PAD, C], f32)
    partial = nc.dram_tensor("partial", [PTOT, C], f32)
    rs_out = nc.dram_tensor("rs_out", [DSLICE_PAD, C], f32)
    a2a_out = nc.dram_tensor("a2a_out", [PTOT, C], f32)

    def batched_rows_ap(handle, r0, nsub):
        # [p, s, ch] view of rows [r0, r0 + nsub*128) of a [rows, C] tensor
        return bass.AP(handle, r0 * C, [[C, P], [P * C, nsub], [1, C]])

    with tile.TileContext(nc) as tc:
        with (
            tc.tile_pool(name="resident", bufs=1) as rpool,
            tc.tile_pool(name="hid", bufs=3) as hpool,
            tc.tile_pool(name="hstage", bufs=4) as opool,
            tc.tile_pool(name="gather", bufs=3) as gpool,
            tc.tile_pool(name="onehot", bufs=3) as spool,
            tc.tile_pool(name="flush", bufs=4) as fpool,
            tc.tile_pool(name="fin", bufs=4) as finpool,
            tc.tile_pool(name="psA", bufs=2, space="PSUM") as psA,
            tc.tile_pool(name="psB", bufs=3, space="PSUM") as psB,
            tc.tile_pool(name="win", bufs=2, space="PSUM") as wpool,
        ):
            # ---- phase 0: resident loads
            w1_sb = [rpool.tile([P, P], f32, tag=f"w1_{i}", name=f"w1sb{i}") for i in range(4)]
            b1_sb = [rpool.tile([P, 1], f32, tag=f"b1_{i}", name=f"b1sb{i}") for i in range(4)]
            w2_sb = [rpool.tile([P, P], f32, tag=f"w2r_{i}", name=f"w2sb{i}") for i in range(4)]
            b2_sb = [rpool.tile([P, P], f32, tag=f"b2r_{i}", name=f"b2sb{i}") for i in range(4)]
            w2s_sb = [rpool.tile([P, P], f32, tag=f"w2s_{i}", name=f"w2ssb{i}") for i in range(4)]
            b2s_sb = [rpool.tile([P, P], f32, tag=f"b2s_{i}", name=f"b2ssb{i}") for i in range(4)]
            emb_sb = [rpool.tile([P, SLICE_PAD], f32, tag=f"emb_{i}", name=f"embsb{i}") for i in range(3)]
            coef_sb = rpool.tile([1, 4], f32, tag="coef")
            coefb_sb = rpool.tile([P, 4], f32, tag="coefb")
            iota_sb = rpool.tile([P, P], f32, tag="iota")
            gidx_sb = rpool.tile([P, B * 256], mybir.dt.int16, tag="gidx")
            slots_sb = rpool.tile([P, T_total], f32, tag="slots")

            for i in range(4):
                nc.sync.dma_start(out=w1_sb[i][:], in_=w1[i][:, :])
                nc.sync.dma_start(out=b1_sb[i][:], in_=b1[i][:, :])
                nc.sync.dma_start(out=w2_sb[i][:], in_=w2[i][:, :])
                nc.sync.dma_start(out=b2_sb[i][:], in_=b2[i][:, :])
            for i in range(3):
                nc.sync.dma_start(out=emb_sb[i][:], in_=embp[i][:, :])
            nc.sync.dma_start(out=coef_sb[:], in_=coef[:, :])
            nc.sync.dma_start(out=iota_sb[:], in_=iota_p[:, :])
            nc.sync.dma_start(out=gidx_sb[:], in_=gidx_p[:, :])
            nc.sync.dma_start(out=slots_sb[:], in_=slots_p[:, :])
            nc.gpsimd.partition_broadcast(coefb_sb[:], coef_sb[:])
            for i in range(4):
                nc.vector.tensor_tensor(
                    out=w2s_sb[i][:], in0=w2_sb[i][:],
                    in1=coefb_sb[:, i : i + 1].to_broadcast([P, P]),
                    op=mybir.AluOpType.mult,
                )
                nc.vector.tensor_tensor(
                    out=b2s_sb[i][:], in0=b2_sb[i][:],
                    in1=coefb_sb[:, i : i + 1].to_broadcast([P, P]),
                    op=mybir.AluOpType.mult,
                )

            # ---- phase 1: MLP tables (hops 1..3 from embA/B/C; self from embA)
            CH = 512
            n_chunks = math.ceil(SLICE_PAD / CH)
            for t in range(4):
                src = emb_sb[min(t, 2)] if t < 3 else emb_sb[0]
                dst = h_dram if t < 3 else self_dram
                row0 = t * SLICE_PAD if t < 3 else 0
                for j in range(n_chunks):
                    c0 = j * CH
                    cw = min(CH, SLICE_PAD - c0)
                    nsub = cw // P
                    ps1 = psA.tile([P, CH], f32, tag="l1", name="ps1")
                    nc.tensor.matmul(
                        ps1[:, :cw], w1_sb[t][:], src[:, c0 : c0 + cw],
                        start=True, stop=True,
                    )
                    u_sb = hpool.tile([P, CH], f32, tag="u", name="u")
                    nc.scalar.activation(
                        u_sb[:, :cw], ps1[:, :cw],
                        mybir.ActivationFunctionType.Relu, bias=b1_sb[t][:],
                    )
                    st = opool.tile([P, 4, P], f32, tag="hst", name="hst")
                    for sub in range(nsub):
                        ps2 = psB.tile([P, P], f32, tag="l2", name="ps2")
                        nc.tensor.matmul(
                            ps2[:], u_sb[:, sub * P : (sub + 1) * P], w2s_sb[t][:],
                            start=True, stop=True,
                        )
                        nc.vector.tensor_tensor(
                            out=st[:, sub, :], in0=ps2[:], in1=b2s_sb[t][:],
                            op=mybir.AluOpType.add,
                        )
                    nc.sync.dma_start(
                        out=batched_rows_ap(dst, row0 + c0, nsub),
                        in_=st[:, :nsub, :],
                    )

            tc.strict_bb_all_engine_barrier()

            def debug_out(srct):
                for j in range(math.ceil(SLICE / P)):
                    r0 = j * P
                    rw = min(P, SLICE - r0)
                    d_sb = finpool.tile([P, C], f32, tag="dbg", name="dbg")
                    nc.sync.dma_start(out=d_sb[:], in_=srct[r0 : r0 + P, :])
                    nc.sync.dma_start(out=out_ext[r0 : r0 + rw, :], in_=d_sb[:rw, :])
                t_sb = finpool.tile([1, 1], f32, tag="tok", name="tok2")
                nc.sync.dma_start(out=t_sb[:], in_=tok_p[:, :])
                nc.sync.dma_start(out=tok_out[:, :], in_=t_sb[:])

            run_p2 = debug_phases >= 2
            run_p3 = debug_phases >= 3
            run_p4 = debug_phases >= 4
            if not run_p2:
                debug_out(h_dram)

            # ---- phase 2: gather + one-hot matmul accumulate, quad-window PSUM
            ps_q = None
            s_blk = None
            for b in range(B if run_p2 else 0):
                gbuf = gpool.tile([P, GBLK, C], f32, tag="g", name="g")
                nc.gpsimd.dma_gather(
                    gbuf[:], h_dram.ap(),
                    gidx_sb[:, b * 256 : (b + 1) * 256],
                    GBLK * P, GBLK * P, C, single_packet=False,
                )
                for tj in range(GBLK):
                    t_idx = b * GBLK + tj
                    if t_idx >= T_total:
                        break
                    if t_idx % SGRP == 0:
                        n_in_grp = min(SGRP, T_total - t_idx)
                        s_blk = spool.tile([P, SGRP, P], f32, tag="s", name="sblk")
                        nc.vector.tensor_tensor(
                            out=s_blk[:, :n_in_grp, :],
                            in0=slots_sb[:, t_idx : t_idx + n_in_grp]
                            .unsqueeze(2).broadcast_to([P, n_in_grp, P]),
                            in1=iota_sb[:].unsqueeze(1).broadcast_to([P, n_in_grp, P]),
                            op=mybir.AluOpType.is_equal,
                        )
                    w = tile_window[t_idx]
                    q, wi = w // WQ, w % WQ
                    first = t_idx == 0 or tile_window[t_idx - 1] != w
                    last = t_idx == T_total - 1 or tile_window[t_idx + 1] != w
                    q_first = first and (wi == 0 or tile_window[t_idx - 1] // WQ != q)
                    q_last = last and (
                        t_idx == T_total - 1 or tile_window[t_idx + 1] // WQ != q
                    )
                    if q_first:
                        ps_q = wpool.tile([P, WQ * P], f32, tag="w", name="psq")
                    nc.tensor.matmul(
                        ps_q[:, wi * P : (wi + 1) * P],
                        s_blk[:, t_idx % SGRP, :],
                        gbuf[:, tj, :],
                        start=first, stop=last,
                    )
                    if q_last:
                        f_sb = fpool.tile([P, WQ, P], f32, tag="f", name="fsb")
                        nc.scalar.copy(f_sb[:], ps_q[:].rearrange("p (s c) -> p s c", s=WQ))
                        nc.sync.dma_start(
                            out=batched_rows_ap(partial, q * WQ * P, WQ),
                            in_=f_sb[:],
                        )

            tc.strict_bb_all_engine_barrier()

            if run_p2 and not run_p3:
                debug_out(partial)

            # ---- phase 3+4: combine partials across cores, add self, write out
            n_fin = math.ceil(SLICE / P)
            if run_p3 and comm_mode == "a2a":
                nc.gpsimd.collective_compute(
                    "AllToAll", mybir.AluOpType.bypass,
                    replica_groups=[list(range(NCORES))],
                    ins=[partial.ap()], outs=[a2a_out.ap()],
                )
                tc.strict_bb_all_engine_barrier()
                if not run_p4:
                    debug_out(a2a_out)
                for j in range(n_fin if run_p4 else 0):
                    r0 = j * P
                    rw = min(P, SLICE - r0)
                    r_sb = finpool.tile([P, NCORES, P], f32, tag="fa", name="fa")
                    # gather the 8 per-core partial copies of this row-tile
                    nc.sync.dma_start(
                        out=r_sb[:],
                        in_=bass.AP(a2a_out, r0 * C, [[C, P], [DSLICE_PAD * C, NCORES], [1, C]]),
                    )
                    s_sb = finpool.tile([P, C], f32, tag="fb", name="fb")
                    nc.sync.dma_start(out=s_sb[:], in_=self_dram[r0 : r0 + P, :])
                    nc.vector.tensor_tensor(
                        out=r_sb[:, 0:4, :], in0=r_sb[:, 0:4, :], in1=r_sb[:, 4:8, :],
                        op=mybir.AluOpType.add,
                    )
                    nc.vector.tensor_tensor(
                        out=r_sb[:, 0:2, :], in0=r_sb[:, 0:2, :], in1=r_sb[:, 2:4, :],
                        op=mybir.AluOpType.add,
                    )
                    nc.vector.tensor_tensor(
                        out=r_sb[:, 0, :], in0=r_sb[:, 0, :], in1=r_sb[:, 1, :],
                        op=mybir.AluOpType.add,
                    )
                    o_sb = finpool.tile([P, C], f32, tag="fo", name="fo")
                    nc.vector.tensor_tensor(
                        out=o_sb[:], in0=r_sb[:, 0, :], in1=s_sb[:],
                        op=mybir.AluOpType.add,
                    )
                    nc.sync.dma_start(out=out_ext[r0 : r0 + rw, :], in_=o_sb[:rw, :])
            elif run_p3:
                nc.gpsimd.collective_compute(
                    "ReduceScatter", mybir.AluOpType.add,
                    replica_groups=[list(range(NCORES))],
                    ins=[partial.ap()], outs=[rs_out.ap()],
                )
                tc.strict_bb_all_engine_barrier()
                if not run_p4:
                    debug_out(rs_out)
                for j in range(n_fin if run_p4 else 0):
                    r0 = j * P
                    rw = min(P, SLICE - r0)
                    a_sb = finpool.tile([P, C], f32, tag="fa", name="fa2")
                    b_sb = finpool.tile([P, C], f32, tag="fb", name="fb2")
                    nc.sync.dma_start(out=a_sb[:], in_=rs_out[r0 : r0 + P, :])
                    nc.sync.dma_start(out=b_sb[:], in_=self_dram[r0 : r0 + P, :])
                    o_sb = finpool.tile([P, C], f32, tag="fo", name="fo2")
                    nc.vector.tensor_tensor(
                        out=o_sb[:], in0=a_sb[:], in1=b_sb[:], op=mybir.AluOpType.add
                    )
                    nc.sync.dma_start(out=out_ext[r0 : r0 + rw, :], in_=o_sb[:rw, :])

            if run_p4:
                t_sb = finpool.tile([1, 1], f32, tag="tok", name="tokf")
                nc.sync.dma_start(out=t_sb[:], in_=tok_p[:, :])
                nc.sync.dma_start(out=tok_out[:, :], in_=t_sb[:])

    nc.compile()
    return nc


# ---------------------------------------------------------------- entry


def make_in_maps(cfg, inputs):
    """Full problem inputs -> per-core in_maps (+ meta)."""
    N, SLICE, SLICE_PAD = cfg["N"], cfg["SLICE"], cfg["SLICE_PAD"]
    ne = np.asarray(inputs["node_embeddings"], np.float32)
    t = int(inputs["t"])
    assert t == 2 and ne.shape[0] == 3
    ei = np.asarray(inputs["edge_index"])
    ew = np.asarray(inputs["edge_weights"])
    per_core_edges, meta = prep_edges(cfg, ei[0], ei[1], ew)

    # per-hop source embedding layers: hop1 -> ne[t], hop2 -> ne[t-1], hop3 -> ne[t-2]
    layers = [ne[2], ne[1], ne[0]]
    hop_coef = np.asarray(inputs["hop_coef"], np.float32)
    coef_in = np.concatenate([hop_coef, [1.0]]).astype(np.float32)[None, :]
    iota_in = np.broadcast_to(
        np.arange(P, dtype=np.float32)[None, :], (P, P)
    ).copy()

    w_names = [
        ("rel_W1", "rel_b1", "rel_W2", "rel_b2"),
        None,  # hh index 0
        None,  # hh index 1
        ("loop_W1", "loop_b1", "loop_W2", "loop_b2"),
    ]

    def wset(i):
        if i in (1, 2):
            W1 = np.asarray(inputs["hh_W1"][i - 1], np.float32)
            bb1 = np.asarray(inputs["hh_b1"][i - 1], np.float32)
            W2 = np.asarray(inputs["hh_W2"][i - 1], np.float32)
            bb2 = np.asarray(inputs["hh_b2"][i - 1], np.float32)
        else:
            n1, n2, n3, n4 = w_names[i]
            W1 = np.asarray(inputs[n1], np.float32)
            bb1 = np.asarray(inputs[n2], np.float32)
            W2 = np.asarray(inputs[n3], np.float32)
            bb2 = np.asarray(inputs[n4], np.float32)
        return (
            np.ascontiguousarray(W1),
            np.ascontiguousarray(bb1[:, None]),
            np.ascontiguousarray(W2),
            np.broadcast_to(bb2[None, :], (P, P)).copy(),
        )

    wsets = [wset(i) for i in range(4)]

    in_maps = []
    for c in range(NCORES):
        m = {}
        for li, name in enumerate(("embA", "embB", "embC")):
            sl = layers[li][c * SLICE : (c + 1) * SLICE]
            pad = np.zeros((P, SLICE_PAD), np.float32)
            pad[:, : sl.shape[0]] = sl.T
            m[name] = pad
        for i in range(4):
            W1, bb1, W2, bb2 = wsets[i]
            m[f"w1_{i}"] = W1
            m[f"b1_{i}"] = bb1
            m[f"w2_{i}"] = W2
            m[f"b2_{i}"] = bb2
        m["coef"] = coef_in
        m["iota"] = iota_in
        m["gidx"] = per_core_edges[c]["gidx"]
        m["slots"] = per_core_edges[c]["slots"]
        m["tok"] = np.zeros((1, 1), np.float32)
        in_maps.append(m)
    return in_maps, meta


def kernel(**inputs):
    ei = np.asarray(inputs["edge_index"])
    ne = np.asarray(inputs["node_embeddings"])
    cfg = make_cfg(ne.shape[1], ei.shape[1])
    in_maps, meta = make_in_maps(cfg, inputs)
    nc = build_kernel(cfg, meta)
    res = run_bass_kernel_spmd(nc, in_maps, core_ids=list(range(NCORES)))
    out = np.concatenate([res.results[c]["out"] for c in range(NCORES)], axis=0)
    return out.astype(np.float32)



# revision 10
# speedup vs baseline: 1.1002x; 1.1002x over previous
"""DRew-GIN layer on 8 TRN2 NeuronCores.

Strategy (source-sharded, no table replication):
  - Nodes are sharded 8 ways. Core c computes the three coef-scaled hop MLP
    tables h'_k = hop_coef[k-1] * MLP_k(emb_src_k) for its node slice only
    (plus the self-loop MLP for its slice), node-major f32 in local DRAM.
  - Edges are partitioned by SOURCE core. Each core produces a PARTIAL
    aggregate over the full (padded) destination range: edges are sorted by
    destination window (128 dest rows); per 128-edge tile we dma_gather the
    source rows from the local h' table, build a one-hot selection matrix
    S^T[e, d] = (slot[e] == d) on DVE, and matmul-accumulate S^T.T @ G into
    the window's PSUM tile. Window flushes go to a partial table
    [8*DSLICE_PAD, 128] f32.
  - One ReduceScatter (CCE add) sums the 8 partials and hands core c its own
    destination slice; add the self-term and write the output slice.

Everything is f32 end-to-end (gather rows are 512B = DMA line-rate minimum).
"""

import math
import sys

sys.path.insert(0, "/opt/trn_rl_repo")

import numpy as np

import concourse.bacc as bacc
import concourse.bass as bass
import concourse.tile as tile
from concourse import mybir
from concourse.bass_utils import run_bass_kernel_spmd

NCORES = 8
C = 128
P = 128
GBLK = 32  # gather block = 32 tiles = 4096 indices


def make_cfg(n_nodes, n_edges):
    assert n_nodes % NCORES == 0
    slice_ = n_nodes // NCORES
    slice_pad = ((slice_ + P - 1) // P) * P
    wps = slice_pad // P  # windows per dest slice
    cfg = dict(
        N=n_nodes,
        E=n_edges,
        SLICE=slice_,
        SLICE_PAD=slice_pad,
        TBL=3 * slice_pad,  # h' table rows per core
        DSLICE_PAD=slice_pad,
        WINDOWS=NCORES * wps,
        WPS=wps,
    )
    return cfg


# ---------------------------------------------------------------- host prep


def prep_edges(cfg, row, col, ew):
    """Returns (per_core {gidx,slots}, meta {tile_window, T_total, B})."""
    N, SLICE, SLICE_PAD, DSLICE_PAD, WINDOWS = (
        cfg["N"],
        cfg["SLICE"],
        cfg["SLICE_PAD"],
        cfg["DSLICE_PAD"],
        cfg["WINDOWS"],
    )
    row = row.astype(np.int64)
    col = col.astype(np.int64)
    ew = ew.astype(np.int64)
    s = col // SLICE
    local = col - s * SLICE
    trow = (ew - 1) * SLICE_PAD + local
    assert trow.max() < 3 * SLICE_PAD <= 32767
    dp = (row // SLICE) * DSLICE_PAD + (row % SLICE)
    w = dp // P
    slot = dp % P

    key = s * WINDOWS + w
    order = np.argsort(key, kind="stable")
    key_s = key[order]
    counts = np.bincount(key_s, minlength=NCORES * WINDOWS).reshape(NCORES, WINDOWS)
    tw = np.maximum(1, (counts.max(axis=0) + P - 1) // P)  # [WINDOWS]
    T_total = int(tw.sum())
    B = (T_total + GBLK - 1) // GBLK
    tile_window = np.repeat(np.arange(WINDOWS), tw)  # [T_total]
    win_tile_off = np.concatenate([[0], np.cumsum(tw)])[:-1]  # [WINDOWS]

    # position of each (sorted) edge inside its (core, window) group
    group_starts = np.concatenate([[0], np.cumsum(counts.reshape(-1))])[:-1]
    pos_in_group = np.arange(len(key_s)) - group_starts[key_s]
    # destination slot index in the padded per-core stream
    core_of = key_s // WINDOWS
    win_of = key_s % WINDOWS
    stream_pos = win_tile_off[win_of] * P + pos_in_group

    NPAD = T_total * P
    per_core = []
    trow_s = trow[order]
    slot_s = slot[order]
    for c in range(NCORES):
        m = core_of == c
        gidx = np.zeros(NPAD, np.int16)  # dummy -> row 0
        slots = np.full(NPAD, 255, np.float32)  # dummy -> no slot match
        gidx[stream_pos[m]] = trow_s[m].astype(np.int16)
        slots[stream_pos[m]] = slot_s[m].astype(np.float32)
        # wrap gidx for dma_gather: block b, idx j -> [j%16, b*256 + j//16]
        blk = np.zeros(B * GBLK * P, np.int16)
        blk[:NPAD] = gidx
        blk = blk.reshape(B, GBLK * P // 16, 16).transpose(0, 2, 1)  # [B,16,256]
        blk = np.tile(blk, (1, 8, 1))  # [B,128,256]
        gidx_in = blk.transpose(1, 0, 2).reshape(P, -1).copy()  # [128, B*256]
        slots_in = slots.reshape(T_total, P).T.copy()  # [128, T_total]
        per_core.append(dict(gidx=gidx_in, slots=slots_in))
    meta = dict(tile_window=tile_window.tolist(), T_total=T_total, B=B)
    return per_core, meta


# ---------------------------------------------------------------- builder


def build_kernel(cfg, meta, debug_phases=4, comm_mode="a2a"):
    SLICE, SLICE_PAD, TBL, DSLICE_PAD, WINDOWS = (
        cfg["SLICE"],
        cfg["SLICE_PAD"],
        cfg["TBL"],
        cfg["DSLICE_PAD"],
        cfg["WINDOWS"],
    )
    T_total, B, tile_window = meta["T_total"], meta["B"], meta["tile_window"]
    PTOT = NCORES * DSLICE_PAD
    f32 = mybir.dt.float32
    bf16 = mybir.dt.bfloat16
    SGRP = 8          # one-hot compare batch (tiles per DVE op)
    WQ = 4            # windows per PSUM bank (quad-window flush)
    assert WINDOWS % WQ == 0

    nc = bacc.Bacc()

    def param(name, shape, dt=f32):
        return nc.declare_dram_parameter(name, list(shape), dt, isOutput=False)

    embp = [param(n, [P, SLICE_PAD]) for n in ("embA", "embB", "embC")]
    w1 = [param(f"w1_{i}", [P, P]) for i in range(4)]  # rel, hh0, hh1, loop
    b1 = [param(f"b1_{i}", [P, 1]) for i in range(4)]
    w2 = [param(f"w2_{i}", [P, P]) for i in range(4)]
    b2 = [param(f"b2_{i}", [P, P]) for i in range(4)]  # row-broadcast
    coef = param("coef", [1, 4])  # hop coefs (4th = 1.0 for self)
    iota_p = param("iota", [P, P])
    gidx_p = param("gidx", [P, B * 256], mybir.dt.int16)
    slots_p = param("slots", [P, T_total])
    tok_p = param("tok", [1, 1])
    out_ext = nc.declare_dram_parameter("out", [SLICE, C], f32, isOutput=True)
    tok_out = nc.declare_dram_parameter("tok_out", [1, 1], f32, isOutput=True)

    h_dram = nc.dram_tensor("h_tbl", [TBL, C], bf16)
    self_dram = nc.dram_tensor("self_tbl", [SLICE_PAD, C], f32)
    partial = nc.dram_tensor("partial", [PTOT, C], bf16)
    rs_out = nc.dram_tensor("rs_out", [DSLICE_PAD, C], bf16)
    a2a_out = nc.dram_tensor("a2a_out", [PTOT, C], bf16)

    def batched_rows_ap(handle, r0, nsub):
        # [p, s, ch] view of rows [r0, r0 + nsub*128) of a [rows, C] tensor
        return bass.AP(handle, r0 * C, [[C, P], [P * C, nsub], [1, C]])

    with tile.TileContext(nc) as tc:
        with (
            tc.tile_pool(name="resident", bufs=1) as rpool,
            tc.tile_pool(name="hid", bufs=3) as hpool,
            tc.tile_pool(name="hstage", bufs=4) as opool,
            tc.tile_pool(name="gather", bufs=3) as gpool,
            tc.tile_pool(name="onehot", bufs=3) as spool,
            tc.tile_pool(name="flush", bufs=4) as fpool,
            tc.tile_pool(name="fin", bufs=4) as finpool,
            tc.tile_pool(name="psA", bufs=2, space="PSUM") as psA,
            tc.tile_pool(name="psB", bufs=3, space="PSUM") as psB,
            tc.tile_pool(name="win", bufs=2, space="PSUM") as wpool,
        ):
            # ---- phase 0: resident loads
            w1_sb = [rpool.tile([P, P], f32, tag=f"w1_{i}", name=f"w1sb{i}") for i in range(4)]
            b1_sb = [rpool.tile([P, 1], f32, tag=f"b1_{i}", name=f"b1sb{i}") for i in range(4)]
            w2_sb = [rpool.tile([P, P], f32, tag=f"w2r_{i}", name=f"w2sb{i}") for i in range(4)]
            b2_sb = [rpool.tile([P, P], f32, tag=f"b2r_{i}", name=f"b2sb{i}") for i in range(4)]
            w2s_sb = [rpool.tile([P, P], f32, tag=f"w2s_{i}", name=f"w2ssb{i}") for i in range(4)]
            b2s_sb = [rpool.tile([P, P], f32, tag=f"b2s_{i}", name=f"b2ssb{i}") for i in range(4)]
            emb_sb = [rpool.tile([P, SLICE_PAD], f32, tag=f"emb_{i}", name=f"embsb{i}") for i in range(3)]
            coef_sb = rpool.tile([1, 4], f32, tag="coef")
            coefb_sb = rpool.tile([P, 4], f32, tag="coefb")
            iota_sb = rpool.tile([P, P], f32, tag="iota")
            gidx_sb = rpool.tile([P, B * 256], mybir.dt.int16, tag="gidx")
            slots_sb = rpool.tile([P, T_total], f32, tag="slots")

            for i in range(4):
                nc.sync.dma_start(out=w1_sb[i][:], in_=w1[i][:, :])
                nc.sync.dma_start(out=b1_sb[i][:], in_=b1[i][:, :])
                nc.sync.dma_start(out=w2_sb[i][:], in_=w2[i][:, :])
                nc.sync.dma_start(out=b2_sb[i][:], in_=b2[i][:, :])
            for i in range(3):
                nc.sync.dma_start(out=emb_sb[i][:], in_=embp[i][:, :])
            nc.sync.dma_start(out=coef_sb[:], in_=coef[:, :])
            nc.sync.dma_start(out=iota_sb[:], in_=iota_p[:, :])
            nc.sync.dma_start(out=gidx_sb[:], in_=gidx_p[:, :])
            nc.sync.dma_start(out=slots_sb[:], in_=slots_p[:, :])
            nc.gpsimd.partition_broadcast(coefb_sb[:], coef_sb[:])
            for i in range(4):
                nc.vector.tensor_tensor(
                    out=w2s_sb[i][:], in0=w2_sb[i][:],
                    in1=coefb_sb[:, i : i + 1].to_broadcast([P, P]),
                    op=mybir.AluOpType.mult,
                )
                nc.vector.tensor_tensor(
                    out=b2s_sb[i][:], in0=b2_sb[i][:],
                    in1=coefb_sb[:, i : i + 1].to_broadcast([P, P]),
                    op=mybir.AluOpType.mult,
                )

            # ---- phase 1: MLP tables (hops 1..3 from embA/B/C; self from embA)
            CH = 512
            n_chunks = math.ceil(SLICE_PAD / CH)
            for t in range(4):
                src = emb_sb[min(t, 2)] if t < 3 else emb_sb[0]
                dst = h_dram if t < 3 else self_dram
                row0 = t * SLICE_PAD if t < 3 else 0
                for j in range(n_chunks):
                    c0 = j * CH
                    cw = min(CH, SLICE_PAD - c0)
                    nsub = cw // P
                    ps1 = psA.tile([P, CH], f32, tag="l1", name="ps1")
                    nc.tensor.matmul(
                        ps1[:, :cw], w1_sb[t][:], src[:, c0 : c0 + cw],
                        start=True, stop=True,
                    )
                    u_sb = hpool.tile([P, CH], f32, tag="u", name="u")
                    nc.scalar.activation(
                        u_sb[:, :cw], ps1[:, :cw],
                        mybir.ActivationFunctionType.Relu, bias=b1_sb[t][:],
                    )
                    st_dt = f32 if t == 3 else bf16
                    st = opool.tile([P, 4, P], st_dt, tag=f"hst{t == 3}", name="hst")
                    for sub in range(nsub):
                        ps2 = psB.tile([P, P], f32, tag="l2", name="ps2")
                        nc.tensor.matmul(
                            ps2[:], u_sb[:, sub * P : (sub + 1) * P], w2s_sb[t][:],
                            start=True, stop=True,
                        )
                        nc.vector.tensor_tensor(
                            out=st[:, sub, :], in0=ps2[:], in1=b2s_sb[t][:],
                            op=mybir.AluOpType.add,
                        )
                    nc.sync.dma_start(
                        out=batched_rows_ap(dst, row0 + c0, nsub),
                        in_=st[:, :nsub, :],
                    )

            tc.strict_bb_all_engine_barrier()

            def debug_out(srct, dt=f32):
                for j in range(math.ceil(SLICE / P)):
                    r0 = j * P
                    rw = min(P, SLICE - r0)
                    d_sb = finpool.tile([P, C], dt, tag="dbg", name="dbg")
                    nc.sync.dma_start(out=d_sb[:], in_=srct[r0 : r0 + P, :])
                    if dt != f32:
                        d32 = finpool.tile([P, C], f32, tag="dbg32", name="dbg32")
                        nc.scalar.copy(d32[:], d_sb[:])
                        d_sb = d32
                    nc.sync.dma_start(out=out_ext[r0 : r0 + rw, :], in_=d_sb[:rw, :])
                t_sb = finpool.tile([1, 1], f32, tag="tok", name="tok2")
                nc.sync.dma_start(out=t_sb[:], in_=tok_p[:, :])
                nc.sync.dma_start(out=tok_out[:, :], in_=t_sb[:])

            run_p2 = debug_phases >= 2
            run_p3 = debug_phases >= 3
            run_p4 = debug_phases >= 4
            if not run_p2:
                debug_out(h_dram, bf16)

            # ---- phase 2: gather + one-hot matmul accumulate, quad-window PSUM
            ps_q = None
            s_blk = None
            for b in range(B if run_p2 else 0):
                gbuf = gpool.tile([P, GBLK, C], bf16, tag="g", name="g")
                nc.gpsimd.dma_gather(
                    gbuf[:], h_dram.ap(),
                    gidx_sb[:, b * 256 : (b + 1) * 256],
                    GBLK * P, GBLK * P, C, single_packet=False,
                )
                for tj in range(GBLK):
                    t_idx = b * GBLK + tj
                    if t_idx >= T_total:
                        break
                    if t_idx % SGRP == 0:
                        n_in_grp = min(SGRP, T_total - t_idx)
                        s_blk = spool.tile([P, SGRP, P], bf16, tag="s", name="sblk")
                        nc.vector.tensor_tensor(
                            out=s_blk[:, :n_in_grp, :],
                            in0=slots_sb[:, t_idx : t_idx + n_in_grp]
                            .unsqueeze(2).broadcast_to([P, n_in_grp, P]),
                            in1=iota_sb[:].unsqueeze(1).broadcast_to([P, n_in_grp, P]),
                            op=mybir.AluOpType.is_equal,
                        )
                    w = tile_window[t_idx]
                    q, wi = w // WQ, w % WQ
                    first = t_idx == 0 or tile_window[t_idx - 1] != w
                    last = t_idx == T_total - 1 or tile_window[t_idx + 1] != w
                    q_first = first and (wi == 0 or tile_window[t_idx - 1] // WQ != q)
                    q_last = last and (
                        t_idx == T_total - 1 or tile_window[t_idx + 1] // WQ != q
                    )
                    if q_first:
                        ps_q = wpool.tile([P, WQ * P], f32, tag="w", name="psq")
                    nc.tensor.matmul(
                        ps_q[:, wi * P : (wi + 1) * P],
                        s_blk[:, t_idx % SGRP, :],
                        gbuf[:, tj, :],
                        start=first, stop=last,
                    )
                    if q_last:
                        f_sb = fpool.tile([P, WQ, P], bf16, tag="f", name="fsb")
                        nc.scalar.copy(f_sb[:], ps_q[:].rearrange("p (s c) -> p s c", s=WQ))
                        nc.sync.dma_start(
                            out=batched_rows_ap(partial, q * WQ * P, WQ),
                            in_=f_sb[:],
                        )

            tc.strict_bb_all_engine_barrier()

            if run_p2 and not run_p3:
                debug_out(partial, bf16)

            # ---- phase 3+4: combine partials across cores, add self, write out
            n_fin = math.ceil(SLICE / P)
            if run_p3 and comm_mode == "a2a":
                nc.gpsimd.collective_compute(
                    "AllToAll", mybir.AluOpType.bypass,
                    replica_groups=[list(range(NCORES))],
                    ins=[partial.ap()], outs=[a2a_out.ap()],
                )
                tc.strict_bb_all_engine_barrier()
                if not run_p4:
                    debug_out(a2a_out, bf16)
                for j in range(n_fin if run_p4 else 0):
                    r0 = j * P
                    rw = min(P, SLICE - r0)
                    r_sb = finpool.tile([P, NCORES, P], bf16, tag="fa", name="fa")
                    # 8 contiguous per-slice reads (one per source core's copy
                    # of this row-tile) — keeps each DMA a bulk linear read
                    for s in range(NCORES):
                        nc.sync.dma_start(
                            out=r_sb[:, s, :],
                            in_=a2a_out[s * DSLICE_PAD + r0 : s * DSLICE_PAD + r0 + P, :],
                        )
                    s_sb = finpool.tile([P, C], f32, tag="fb", name="fb")
                    nc.sync.dma_start(out=s_sb[:], in_=self_dram[r0 : r0 + P, :])
                    nc.vector.tensor_tensor(
                        out=r_sb[:, 0:4, :], in0=r_sb[:, 0:4, :], in1=r_sb[:, 4:8, :],
                        op=mybir.AluOpType.add,
                    )
                    nc.vector.tensor_tensor(
                        out=r_sb[:, 0:2, :], in0=r_sb[:, 0:2, :], in1=r_sb[:, 2:4, :],
                        op=mybir.AluOpType.add,
                    )
                    h32 = finpool.tile([P, C], f32, tag="fh", name="fh")
                    nc.vector.tensor_tensor(
                        out=h32[:], in0=r_sb[:, 0, :], in1=r_sb[:, 1, :],
                        op=mybir.AluOpType.add,
                    )
                    o_sb = finpool.tile([P, C], f32, tag="fo", name="fo")
                    nc.vector.tensor_tensor(
                        out=o_sb[:], in0=h32[:], in1=s_sb[:],
                        op=mybir.AluOpType.add,
                    )
                    nc.sync.dma_start(out=out_ext[r0 : r0 + rw, :], in_=o_sb[:rw, :])
            elif run_p3:
                nc.gpsimd.collective_compute(
                    "ReduceScatter", mybir.AluOpType.add,
                    replica_groups=[list(range(NCORES))],
                    ins=[partial.ap()], outs=[rs_out.ap()],
                )
                tc.strict_bb_all_engine_barrier()
                if not run_p4:
                    debug_out(rs_out, bf16)
                for j in range(n_fin if run_p4 else 0):
                    r0 = j * P
                    rw = min(P, SLICE - r0)
                    a_sb = finpool.tile([P, C], bf16, tag="fa", name="fa2")
                    b_sb = finpool.tile([P, C], f32, tag="fb", name="fb2")
                    nc.sync.dma_start(out=a_sb[:], in_=rs_out[r0 : r0 + P, :])
                    nc.sync.dma_start(out=b_sb[:], in_=self_dram[r0 : r0 + P, :])
                    a32 = finpool.tile([P, C], f32, tag="fh", name="fh2")
                    nc.scalar.copy(a32[:], a_sb[:])
                    o_sb = finpool.tile([P, C], f32, tag="fo", name="fo2")
                    nc.vector.tensor_tensor(
                        out=o_sb[:], in0=a32[:], in1=b_sb[:], op=mybir.AluOpType.add
                    )
                    nc.sync.dma_start(out=out_ext[r0 : r0 + rw, :], in_=o_sb[:rw, :])

            if run_p4:
                t_sb = finpool.tile([1, 1], f32, tag="tok", name="tokf")
                nc.sync.dma_start(out=t_sb[:], in_=tok_p[:, :])
                nc.sync.dma_start(out=tok_out[:, :], in_=t_sb[:])

    nc.compile()
    return nc


# ---------------------------------------------------------------- entry


def make_in_maps(cfg, inputs):
    """Full problem inputs -> per-core in_maps (+ meta)."""
    N, SLICE, SLICE_PAD = cfg["N"], cfg["SLICE"], cfg["SLICE_PAD"]
    ne = np.asarray(inputs["node_embeddings"], np.float32)
    t = int(inputs["t"])
    assert t == 2 and ne.shape[0] == 3
    ei = np.asarray(inputs["edge_index"])
    ew = np.asarray(inputs["edge_weights"])
    per_core_edges, meta = prep_edges(cfg, ei[0], ei[1], ew)

    # per-hop source embedding layers: hop1 -> ne[t], hop2 -> ne[t-1], hop3 -> ne[t-2]
    layers = [ne[2], ne[1], ne[0]]
    hop_coef = np.asarray(inputs["hop_coef"], np.float32)
    coef_in = np.concatenate([hop_coef, [1.0]]).astype(np.float32)[None, :]
    iota_in = np.broadcast_to(
        np.arange(P, dtype=np.float32)[None, :], (P, P)
    ).copy()

    w_names = [
        ("rel_W1", "rel_b1", "rel_W2", "rel_b2"),
        None,  # hh index 0
        None,  # hh index 1
        ("loop_W1", "loop_b1", "loop_W2", "loop_b2"),
    ]

    def wset(i):
        if i in (1, 2):
            W1 = np.asarray(inputs["hh_W1"][i - 1], np.float32)
            bb1 = np.asarray(inputs["hh_b1"][i - 1], np.float32)
            W2 = np.asarray(inputs["hh_W2"][i - 1], np.float32)
            bb2 = np.asarray(inputs["hh_b2"][i - 1], np.float32)
        else:
            n1, n2, n3, n4 = w_names[i]
            W1 = np.asarray(inputs[n1], np.float32)
            bb1 = np.asarray(inputs[n2], np.float32)
            W2 = np.asarray(inputs[n3], np.float32)
            bb2 = np.asarray(inputs[n4], np.float32)
        return (
            np.ascontiguousarray(W1),
            np.ascontiguousarray(bb1[:, None]),
            np.ascontiguousarray(W2),
            np.broadcast_to(bb2[None, :], (P, P)).copy(),
        )

    wsets = [wset(i) for i in range(4)]

    in_maps = []
    for c in range(NCORES):
        m = {}
        for li, name in enumerate(("embA", "embB", "embC")):
            sl = layers[li][c * SLICE : (c + 1) * SLICE]
            pad = np.zeros((P, SLICE_PAD), np.float32)
            pad[:, : sl.shape[0]] = sl.T
            m[name] = pad
        for i in range(4):
            W1, bb1, W2, bb2 = wsets[i]
            m[f"w1_{i}"] = W1
            m[f"b1_{i}"] = bb1
            m[f"w2_{i}"] = W2
            m[f"b2_{i}"] = bb2
        m["coef"] = coef_in
        m["iota"] = iota_in
        m["gidx"] = per_core_edges[c]["gidx"]
        m["slots"] = per_core_edges[c]["slots"]
        m["tok"] = np.zeros((1, 1), np.float32)
        in_maps.append(m)
    return in_maps, meta


def kernel(**inputs):
    ei = np.asarray(inputs["edge_index"])
    ne = np.asarray(inputs["node_embeddings"])
    cfg = make_cfg(ne.shape[1], ei.shape[1])
    in_maps, meta = make_in_maps(cfg, inputs)
    nc = build_kernel(cfg, meta)
    res = run_bass_kernel_spmd(nc, in_maps, core_ids=list(range(NCORES)))
    out = np.concatenate([res.results[c]["out"] for c in range(NCORES)], axis=0)
    return out.astype(np.float32)



# revision 18
# speedup vs baseline: 1.1503x; 1.0455x over previous
"""DRew-GIN layer on 8 TRN2 NeuronCores.

Strategy (source-sharded, no table replication):
  - Nodes are sharded 8 ways. Core c computes the three coef-scaled hop MLP
    tables h'_k = hop_coef[k-1] * MLP_k(emb_src_k) for its node slice only
    (plus the self-loop MLP for its slice), node-major f32 in local DRAM.
  - Edges are partitioned by SOURCE core. Each core produces a PARTIAL
    aggregate over the full (padded) destination range: edges are sorted by
    destination window (128 dest rows); per 128-edge tile we dma_gather the
    source rows from the local h' table, build a one-hot selection matrix
    S^T[e, d] = (slot[e] == d) on DVE, and matmul-accumulate S^T.T @ G into
    the window's PSUM tile. Window flushes go to a partial table
    [8*DSLICE_PAD, 128] f32.
  - One ReduceScatter (CCE add) sums the 8 partials and hands core c its own
    destination slice; add the self-term and write the output slice.

Everything is f32 end-to-end (gather rows are 512B = DMA line-rate minimum).
"""

import math
import sys

sys.path.insert(0, "/opt/trn_rl_repo")

import ml_dtypes
import numpy as np

import concourse.bacc as bacc
import concourse.bass as bass
import concourse.tile as tile
from concourse import mybir
from concourse.bass_utils import run_bass_kernel_spmd

NCORES = 8
C = 128
P = 128
GBLK = 32  # gather block = 32 tiles = 4096 indices


def make_cfg(n_nodes, n_edges):
    assert n_nodes % NCORES == 0
    slice_ = n_nodes // NCORES
    slice_pad = ((slice_ + P - 1) // P) * P
    wps = slice_pad // P  # windows per dest slice
    cfg = dict(
        N=n_nodes,
        E=n_edges,
        SLICE=slice_,
        SLICE_PAD=slice_pad,
        TBL=3 * slice_pad,  # h' table rows per core
        DSLICE_PAD=slice_pad,
        WINDOWS=NCORES * wps,
        WPS=wps,
    )
    return cfg


# ---------------------------------------------------------------- host prep


def prep_edges(cfg, row, col, ew):
    """Returns (per_core {gidx,slots}, meta {tile_window, T_total, B})."""
    N, SLICE, SLICE_PAD, DSLICE_PAD, WINDOWS = (
        cfg["N"],
        cfg["SLICE"],
        cfg["SLICE_PAD"],
        cfg["DSLICE_PAD"],
        cfg["WINDOWS"],
    )
    row = row.astype(np.int64)
    col = col.astype(np.int64)
    ew = ew.astype(np.int64)
    s = col // SLICE
    local = col - s * SLICE
    trow = (ew - 1) * SLICE_PAD + local
    assert trow.max() < 3 * SLICE_PAD <= 32767
    dp = (row // SLICE) * DSLICE_PAD + (row % SLICE)
    w = dp // P
    slot = dp % P

    key = s * WINDOWS + w
    order = np.argsort(key, kind="stable")
    key_s = key[order]
    counts = np.bincount(key_s, minlength=NCORES * WINDOWS).reshape(NCORES, WINDOWS)
    tw = np.maximum(1, (counts.max(axis=0) + P - 1) // P)  # [WINDOWS]
    T_total = int(tw.sum())
    B = (T_total + GBLK - 1) // GBLK
    tile_window = np.repeat(np.arange(WINDOWS), tw)  # [T_total]
    win_tile_off = np.concatenate([[0], np.cumsum(tw)])[:-1]  # [WINDOWS]

    # position of each (sorted) edge inside its (core, window) group
    group_starts = np.concatenate([[0], np.cumsum(counts.reshape(-1))])[:-1]
    pos_in_group = np.arange(len(key_s)) - group_starts[key_s]
    # destination slot index in the padded per-core stream
    core_of = key_s // WINDOWS
    win_of = key_s % WINDOWS
    stream_pos = win_tile_off[win_of] * P + pos_in_group

    NPAD = T_total * P
    per_core = []
    trow_s = trow[order]
    slot_s = slot[order]
    for c in range(NCORES):
        m = core_of == c
        gidx = np.zeros(NPAD, np.int16)  # dummy -> row 0
        slots = np.full(NPAD, 255, np.float32)  # dummy -> no slot match
        gidx[stream_pos[m]] = trow_s[m].astype(np.int16)
        slots[stream_pos[m]] = slot_s[m].astype(np.float32)
        # wrap gidx for dma_gather: block b, idx j -> [j%16, b*256 + j//16]
        blk = np.zeros(B * GBLK * P, np.int16)
        blk[:NPAD] = gidx
        blk = blk.reshape(B, GBLK * P // 16, 16).transpose(0, 2, 1)  # [B,16,256]
        blk = np.tile(blk, (1, 8, 1))  # [B,128,256]
        gidx_in = blk.transpose(1, 0, 2).reshape(P, -1).copy()  # [128, B*256]
        slots_in = slots.reshape(T_total, P).T.copy()  # [128, T_total]
        per_core.append(dict(gidx=gidx_in, slots=slots_in))
    meta = dict(tile_window=tile_window.tolist(), T_total=T_total, B=B)
    return per_core, meta


# ---------------------------------------------------------------- builder


def build_kernel(cfg, meta, debug_phases=4, comm_mode="a2a"):
    SLICE, SLICE_PAD, TBL, DSLICE_PAD, WINDOWS = (
        cfg["SLICE"],
        cfg["SLICE_PAD"],
        cfg["TBL"],
        cfg["DSLICE_PAD"],
        cfg["WINDOWS"],
    )
    T_total, B, tile_window = meta["T_total"], meta["B"], meta["tile_window"]
    PTOT = NCORES * DSLICE_PAD
    f32 = mybir.dt.float32
    bf16 = mybir.dt.bfloat16
    SGRP = 8          # one-hot compare batch (tiles per DVE op)
    WQ = 4            # windows per PSUM bank (quad-window flush)
    assert WINDOWS % WQ == 0

    nc = bacc.Bacc(num_swdge_queues=4)

    def param(name, shape, dt=f32):
        return nc.declare_dram_parameter(name, list(shape), dt, isOutput=False)

    embp = [param(n, [P, SLICE_PAD], bf16) for n in ("embA", "embB", "embC")]
    w1 = [param(f"w1_{i}", [P, P], bf16) for i in range(4)]  # rel, hh0, hh1, loop
    b1 = [param(f"b1_{i}", [P, 1]) for i in range(4)]
    # W2/b2 pre-scaled by hop_coef on the host
    w2s = [param(f"w2s_{i}", [P, P], bf16) for i in range(4)]
    b2s = [param(f"b2s_{i}", [P, P]) for i in range(4)]  # row-broadcast
    iota_p = param("iota", [P, P])
    gidx_p = param("gidx", [P, B * 256], mybir.dt.int16)
    slots_p = param("slots", [P, T_total])
    tok_p = param("tok", [1, 1])
    out_ext = nc.declare_dram_parameter("out", [SLICE, C], f32, isOutput=True)
    tok_out = nc.declare_dram_parameter("tok_out", [1, 1], f32, isOutput=True)

    h_dram = nc.dram_tensor("h_tbl", [TBL, C], bf16)
    self_dram = nc.dram_tensor("self_tbl", [SLICE_PAD, C], f32)
    partial = nc.dram_tensor("partial", [PTOT, C], bf16)
    rs_out = nc.dram_tensor("rs_out", [DSLICE_PAD, C], bf16)
    a2a_out = nc.dram_tensor("a2a_out", [PTOT, C], bf16)

    def batched_rows_ap(handle, r0, nsub):
        # [p, s, ch] view of rows [r0, r0 + nsub*128) of a [rows, C] tensor
        return bass.AP(handle, r0 * C, [[C, P], [P * C, nsub], [1, C]])

    with tile.TileContext(nc) as tc:
        with (
            tc.tile_pool(name="resident", bufs=1) as rpool,
            tc.tile_pool(name="hid", bufs=3) as hpool,
            tc.tile_pool(name="hstage", bufs=4) as opool,
            tc.tile_pool(name="gather", bufs=4) as gpool,
            tc.tile_pool(name="onehot", bufs=3) as spool,
            tc.tile_pool(name="flush", bufs=4) as fpool,
            tc.tile_pool(name="fin", bufs=4) as finpool,
            tc.tile_pool(name="psA", bufs=2, space="PSUM") as psA,
            tc.tile_pool(name="psB", bufs=3, space="PSUM") as psB,
            tc.tile_pool(name="win", bufs=2, space="PSUM") as wpool,
        ):
            # ---- phase 0: resident loads
            w1_sb = [rpool.tile([P, P], bf16, tag=f"w1_{i}", name=f"w1sb{i}") for i in range(4)]
            b1_sb = [rpool.tile([P, 1], f32, tag=f"b1_{i}", name=f"b1sb{i}") for i in range(4)]
            w2s_sb = [rpool.tile([P, P], bf16, tag=f"w2s_{i}", name=f"w2ssb{i}") for i in range(4)]
            b2s_sb = [rpool.tile([P, P], f32, tag=f"b2s_{i}", name=f"b2ssb{i}") for i in range(4)]
            emb_sb = [rpool.tile([P, SLICE_PAD], bf16, tag=f"emb_{i}", name=f"embsb{i}") for i in range(3)]
            iota_sb = rpool.tile([P, P], f32, tag="iota")
            gidx_sb = rpool.tile([P, B * 256], mybir.dt.int16, tag="gidx")
            slots_sb = rpool.tile([P, T_total], f32, tag="slots")

            for i in range(4):
                nc.sync.dma_start(out=w1_sb[i][:], in_=w1[i][:, :])
                nc.sync.dma_start(out=b1_sb[i][:], in_=b1[i][:, :])
                nc.sync.dma_start(out=w2s_sb[i][:], in_=w2s[i][:, :])
                nc.sync.dma_start(out=b2s_sb[i][:], in_=b2s[i][:, :])
            for i in range(3):
                nc.sync.dma_start(out=emb_sb[i][:], in_=embp[i][:, :])
            nc.sync.dma_start(out=iota_sb[:], in_=iota_p[:, :])
            nc.sync.dma_start(out=gidx_sb[:], in_=gidx_p[:, :])
            nc.sync.dma_start(out=slots_sb[:], in_=slots_p[:, :])

            # ---- phase 1: MLP tables (hops 1..3 from embA/B/C; self from embA)
            CH = 512
            n_chunks = math.ceil(SLICE_PAD / CH)
            for t in range(4):
                src = emb_sb[min(t, 2)] if t < 3 else emb_sb[0]
                dst = h_dram if t < 3 else self_dram
                row0 = t * SLICE_PAD if t < 3 else 0
                for j in range(n_chunks):
                    c0 = j * CH
                    cw = min(CH, SLICE_PAD - c0)
                    nsub = cw // P
                    ps1 = psA.tile([P, CH], f32, tag="l1", name="ps1")
                    nc.tensor.matmul(
                        ps1[:, :cw], w1_sb[t][:], src[:, c0 : c0 + cw],
                        start=True, stop=True,
                    )
                    u_sb = hpool.tile([P, CH], bf16, tag="u", name="u")
                    nc.scalar.activation(
                        u_sb[:, :cw], ps1[:, :cw],
                        mybir.ActivationFunctionType.Relu, bias=b1_sb[t][:],
                    )
                    st_dt = f32 if t == 3 else bf16
                    st = opool.tile([P, 4, P], st_dt, tag=f"hst{t == 3}", name="hst")
                    for sub in range(nsub):
                        ps2 = psB.tile([P, P], f32, tag="l2", name="ps2")
                        nc.tensor.matmul(
                            ps2[:], u_sb[:, sub * P : (sub + 1) * P], w2s_sb[t][:],
                            start=True, stop=True,
                        )
                        nc.vector.tensor_tensor(
                            out=st[:, sub, :], in0=ps2[:], in1=b2s_sb[t][:],
                            op=mybir.AluOpType.add,
                        )
                    nc.sync.dma_start(
                        out=batched_rows_ap(dst, row0 + c0, nsub),
                        in_=st[:, :nsub, :],
                    )

            tc.strict_bb_all_engine_barrier()

            def debug_out(srct, dt=f32):
                for j in range(math.ceil(SLICE / P)):
                    r0 = j * P
                    rw = min(P, SLICE - r0)
                    d_sb = finpool.tile([P, C], dt, tag="dbg", name="dbg")
                    nc.sync.dma_start(out=d_sb[:], in_=srct[r0 : r0 + P, :])
                    if dt != f32:
                        d32 = finpool.tile([P, C], f32, tag="dbg32", name="dbg32")
                        nc.scalar.copy(d32[:], d_sb[:])
                        d_sb = d32
                    nc.sync.dma_start(out=out_ext[r0 : r0 + rw, :], in_=d_sb[:rw, :])
                t_sb = finpool.tile([1, 1], f32, tag="tok", name="tok2")
                nc.sync.dma_start(out=t_sb[:], in_=tok_p[:, :])
                nc.sync.dma_start(out=tok_out[:, :], in_=t_sb[:])

            run_p2 = debug_phases >= 2
            run_p3 = debug_phases >= 3
            run_p4 = debug_phases >= 4
            if not run_p2:
                debug_out(h_dram, bf16)

            # ---- phase 2: gather + one-hot matmul accumulate, quad-window PSUM
            ps_q = None
            s_blk = None
            for b in range(B if run_p2 else 0):
                gbuf = gpool.tile([P, GBLK, C], bf16, tag="g", name="g")
                nc.gpsimd.dma_gather(
                    gbuf[:], h_dram.ap(),
                    gidx_sb[:, b * 256 : (b + 1) * 256],
                    GBLK * P, GBLK * P, C, single_packet=False,
                    queue_num=b % 4,
                )
                for tj in range(GBLK):
                    t_idx = b * GBLK + tj
                    if t_idx >= T_total:
                        break
                    if t_idx % SGRP == 0:
                        n_in_grp = min(SGRP, T_total - t_idx)
                        s_blk = spool.tile([P, SGRP, P], bf16, tag="s", name="sblk")
                        nc.vector.tensor_tensor(
                            out=s_blk[:, :n_in_grp, :],
                            in0=slots_sb[:, t_idx : t_idx + n_in_grp]
                            .unsqueeze(2).broadcast_to([P, n_in_grp, P]),
                            in1=iota_sb[:].unsqueeze(1).broadcast_to([P, n_in_grp, P]),
                            op=mybir.AluOpType.is_equal,
                        )
                    w = tile_window[t_idx]
                    q, wi = w // WQ, w % WQ
                    first = t_idx == 0 or tile_window[t_idx - 1] != w
                    last = t_idx == T_total - 1 or tile_window[t_idx + 1] != w
                    q_first = first and (wi == 0 or tile_window[t_idx - 1] // WQ != q)
                    q_last = last and (
                        t_idx == T_total - 1 or tile_window[t_idx + 1] // WQ != q
                    )
                    if q_first:
                        ps_q = wpool.tile([P, WQ * P], f32, tag="w", name="psq")
                    nc.tensor.matmul(
                        ps_q[:, wi * P : (wi + 1) * P],
                        s_blk[:, t_idx % SGRP, :],
                        gbuf[:, tj, :],
                        start=first, stop=last,
                    )
                    if q_last:
                        f_sb = fpool.tile([P, WQ, P], bf16, tag="f", name="fsb")
                        nc.scalar.copy(f_sb[:], ps_q[:].rearrange("p (s c) -> p s c", s=WQ))
                        nc.sync.dma_start(
                            out=batched_rows_ap(partial, q * WQ * P, WQ),
                            in_=f_sb[:],
                        )

            tc.strict_bb_all_engine_barrier()

            if run_p2 and not run_p3:
                debug_out(partial, bf16)

            # ---- phase 3+4: combine partials across cores, add self, write out
            n_fin = math.ceil(SLICE / P)
            if run_p3 and comm_mode == "a2a":
                nc.gpsimd.collective_compute(
                    "AllToAll", mybir.AluOpType.bypass,
                    replica_groups=[list(range(NCORES))],
                    ins=[partial.ap()], outs=[a2a_out.ap()],
                )
                tc.strict_bb_all_engine_barrier()
                if not run_p4:
                    debug_out(a2a_out, bf16)
                for j in range(n_fin if run_p4 else 0):
                    r0 = j * P
                    rw = min(P, SLICE - r0)
                    r_sb = finpool.tile([P, NCORES, P], bf16, tag="fa", name="fa")
                    # 8 contiguous per-slice reads (one per source core's copy
                    # of this row-tile) — keeps each DMA a bulk linear read
                    for s in range(NCORES):
                        nc.sync.dma_start(
                            out=r_sb[:, s, :],
                            in_=a2a_out[s * DSLICE_PAD + r0 : s * DSLICE_PAD + r0 + P, :],
                        )
                    s_sb = finpool.tile([P, C], f32, tag="fb", name="fb")
                    nc.sync.dma_start(out=s_sb[:], in_=self_dram[r0 : r0 + P, :])
                    nc.vector.tensor_tensor(
                        out=r_sb[:, 0:4, :], in0=r_sb[:, 0:4, :], in1=r_sb[:, 4:8, :],
                        op=mybir.AluOpType.add,
                    )
                    nc.vector.tensor_tensor(
                        out=r_sb[:, 0:2, :], in0=r_sb[:, 0:2, :], in1=r_sb[:, 2:4, :],
                        op=mybir.AluOpType.add,
                    )
                    h32 = finpool.tile([P, C], f32, tag="fh", name="fh")
                    nc.vector.tensor_tensor(
                        out=h32[:], in0=r_sb[:, 0, :], in1=r_sb[:, 1, :],
                        op=mybir.AluOpType.add,
                    )
                    o_sb = finpool.tile([P, C], f32, tag="fo", name="fo")
                    nc.vector.tensor_tensor(
                        out=o_sb[:], in0=h32[:], in1=s_sb[:],
                        op=mybir.AluOpType.add,
                    )
                    nc.sync.dma_start(out=out_ext[r0 : r0 + rw, :], in_=o_sb[:rw, :])
            elif run_p3:
                nc.gpsimd.collective_compute(
                    "ReduceScatter", mybir.AluOpType.add,
                    replica_groups=[list(range(NCORES))],
                    ins=[partial.ap()], outs=[rs_out.ap()],
                )
                tc.strict_bb_all_engine_barrier()
                if not run_p4:
                    debug_out(rs_out, bf16)
                for j in range(n_fin if run_p4 else 0):
                    r0 = j * P
                    rw = min(P, SLICE - r0)
                    a_sb = finpool.tile([P, C], bf16, tag="fa", name="fa2")
                    b_sb = finpool.tile([P, C], f32, tag="fb", name="fb2")
                    nc.sync.dma_start(out=a_sb[:], in_=rs_out[r0 : r0 + P, :])
                    nc.sync.dma_start(out=b_sb[:], in_=self_dram[r0 : r0 + P, :])
                    a32 = finpool.tile([P, C], f32, tag="fh", name="fh2")
                    nc.scalar.copy(a32[:], a_sb[:])
                    o_sb = finpool.tile([P, C], f32, tag="fo", name="fo2")
                    nc.vector.tensor_tensor(
                        out=o_sb[:], in0=a32[:], in1=b_sb[:], op=mybir.AluOpType.add
                    )
                    nc.sync.dma_start(out=out_ext[r0 : r0 + rw, :], in_=o_sb[:rw, :])

            if run_p4:
                t_sb = finpool.tile([1, 1], f32, tag="tok", name="tokf")
                nc.sync.dma_start(out=t_sb[:], in_=tok_p[:, :])
                nc.sync.dma_start(out=tok_out[:, :], in_=t_sb[:])

    nc.compile()
    return nc


# ---------------------------------------------------------------- entry


def make_in_maps(cfg, inputs):
    """Full problem inputs -> per-core in_maps (+ meta)."""
    N, SLICE, SLICE_PAD = cfg["N"], cfg["SLICE"], cfg["SLICE_PAD"]
    ne = np.asarray(inputs["node_embeddings"], np.float32)
    t = int(inputs["t"])
    assert t == 2 and ne.shape[0] == 3
    ei = np.asarray(inputs["edge_index"])
    ew = np.asarray(inputs["edge_weights"])
    per_core_edges, meta = prep_edges(cfg, ei[0], ei[1], ew)

    # per-hop source embedding layers: hop1 -> ne[t], hop2 -> ne[t-1], hop3 -> ne[t-2]
    layers = [ne[2], ne[1], ne[0]]
    hop_coef = np.asarray(inputs["hop_coef"], np.float32)
    coef4 = np.concatenate([hop_coef, [1.0]]).astype(np.float32)
    iota_in = np.broadcast_to(
        np.arange(P, dtype=np.float32)[None, :], (P, P)
    ).copy()

    w_names = [
        ("rel_W1", "rel_b1", "rel_W2", "rel_b2"),
        None,  # hh index 0
        None,  # hh index 1
        ("loop_W1", "loop_b1", "loop_W2", "loop_b2"),
    ]

    def wset(i):
        if i in (1, 2):
            W1 = np.asarray(inputs["hh_W1"][i - 1], np.float32)
            bb1 = np.asarray(inputs["hh_b1"][i - 1], np.float32)
            W2 = np.asarray(inputs["hh_W2"][i - 1], np.float32)
            bb2 = np.asarray(inputs["hh_b2"][i - 1], np.float32)
        else:
            n1, n2, n3, n4 = w_names[i]
            W1 = np.asarray(inputs[n1], np.float32)
            bb1 = np.asarray(inputs[n2], np.float32)
            W2 = np.asarray(inputs[n3], np.float32)
            bb2 = np.asarray(inputs[n4], np.float32)
        # pre-scale second-layer weights/bias by this hop's coefficient
        return (
            np.ascontiguousarray(W1).astype(ml_dtypes.bfloat16),
            np.ascontiguousarray(bb1[:, None]),
            np.ascontiguousarray(W2 * coef4[i]).astype(ml_dtypes.bfloat16),
            np.broadcast_to((bb2 * coef4[i])[None, :], (P, P)).copy(),
        )

    wsets = [wset(i) for i in range(4)]

    in_maps = []
    for c in range(NCORES):
        m = {}
        for li, name in enumerate(("embA", "embB", "embC")):
            sl = layers[li][c * SLICE : (c + 1) * SLICE]
            pad = np.zeros((P, SLICE_PAD), ml_dtypes.bfloat16)
            pad[:, : sl.shape[0]] = sl.T.astype(ml_dtypes.bfloat16)
            m[name] = pad
        for i in range(4):
            W1, bb1, W2s, bb2s = wsets[i]
            m[f"w1_{i}"] = W1
            m[f"b1_{i}"] = bb1
            m[f"w2s_{i}"] = W2s
            m[f"b2s_{i}"] = bb2s
        m["iota"] = iota_in
        m["gidx"] = per_core_edges[c]["gidx"]
        m["slots"] = per_core_edges[c]["slots"]
        m["tok"] = np.zeros((1, 1), np.float32)
        in_maps.append(m)
    return in_maps, meta


def kernel(**inputs):
    ei = np.asarray(inputs["edge_index"])
    ne = np.asarray(inputs["node_embeddings"])
    cfg = make_cfg(ne.shape[1], ei.shape[1])
    in_maps, meta = make_in_maps(cfg, inputs)
    nc = build_kernel(cfg, meta)
    res = run_bass_kernel_spmd(nc, in_maps, core_ids=list(range(NCORES)))
    out = np.concatenate([res.results[c]["out"] for c in range(NCORES)], axis=0)
    return out.astype(np.float32)



# revision 23
# speedup vs baseline: 4.9520x; 4.3048x over previous
"""DRew-GIN layer on 8 TRN2 NeuronCores.

Strategy (source-sharded, no table replication):
  - Nodes are sharded 8 ways. Core c computes the three coef-scaled hop MLP
    tables h'_k = hop_coef[k-1] * MLP_k(emb_src_k) for its node slice only
    (plus the self-loop MLP for its slice), node-major f32 in local DRAM.
  - Edges are partitioned by SOURCE core. Each core produces a PARTIAL
    aggregate over the full (padded) destination range: edges are sorted by
    destination window (128 dest rows); per 128-edge tile we dma_gather the
    source rows from the local h' table, build a one-hot selection matrix
    S^T[e, d] = (slot[e] == d) on DVE, and matmul-accumulate S^T.T @ G into
    the window's PSUM tile. Window flushes go to a partial table
    [8*DSLICE_PAD, 128] f32.
  - One ReduceScatter (CCE add) sums the 8 partials and hands core c its own
    destination slice; add the self-term and write the output slice.

Everything is f32 end-to-end (gather rows are 512B = DMA line-rate minimum).
"""

import math
import sys

sys.path.insert(0, "/opt/trn_rl_repo")

import ml_dtypes
import numpy as np

import concourse.bacc as bacc
import concourse.bass as bass
import concourse.tile as tile
from concourse import mybir
from concourse.bass_utils import run_bass_kernel_spmd

NCORES = 8
C = 128
P = 128
GBLK = 32  # gather block = 32 tiles = 4096 indices


def make_cfg(n_nodes, n_edges):
    assert n_nodes % NCORES == 0
    slice_ = n_nodes // NCORES
    slice_pad = ((slice_ + P - 1) // P) * P
    wps = slice_pad // P  # windows per dest slice
    cfg = dict(
        N=n_nodes,
        E=n_edges,
        SLICE=slice_,
        SLICE_PAD=slice_pad,
        TBL=3 * slice_pad,  # h' table rows per core
        DSLICE_PAD=slice_pad,
        WINDOWS=NCORES * wps,
        WPS=wps,
    )
    return cfg


# ---------------------------------------------------------------- host prep


def prep_edges(cfg, row, col, ew):
    """Returns (per_core {gidx,slots}, meta {tile_window, T_total, B})."""
    N, SLICE, SLICE_PAD, DSLICE_PAD, WINDOWS = (
        cfg["N"],
        cfg["SLICE"],
        cfg["SLICE_PAD"],
        cfg["DSLICE_PAD"],
        cfg["WINDOWS"],
    )
    row = row.astype(np.int64)
    col = col.astype(np.int64)
    ew = ew.astype(np.int64)
    s = col // SLICE
    local = col - s * SLICE
    trow = (ew - 1) * SLICE_PAD + local
    assert trow.max() < 3 * SLICE_PAD <= 32767
    dp = (row // SLICE) * DSLICE_PAD + (row % SLICE)
    w = dp // P
    slot = dp % P

    key = s * WINDOWS + w
    order = np.argsort(key, kind="stable")
    key_s = key[order]
    counts = np.bincount(key_s, minlength=NCORES * WINDOWS).reshape(NCORES, WINDOWS)
    tw = np.maximum(1, (counts.max(axis=0) + P - 1) // P)  # [WINDOWS]
    T_total = int(tw.sum())
    B = (T_total + GBLK - 1) // GBLK
    tile_window = np.repeat(np.arange(WINDOWS), tw)  # [T_total]
    win_tile_off = np.concatenate([[0], np.cumsum(tw)])[:-1]  # [WINDOWS]

    # position of each (sorted) edge inside its (core, window) group
    group_starts = np.concatenate([[0], np.cumsum(counts.reshape(-1))])[:-1]
    pos_in_group = np.arange(len(key_s)) - group_starts[key_s]
    # destination slot index in the padded per-core stream
    core_of = key_s // WINDOWS
    win_of = key_s % WINDOWS
    stream_pos = win_tile_off[win_of] * P + pos_in_group

    NPAD = T_total * P
    per_core = []
    trow_s = trow[order]
    slot_s = slot[order]
    for c in range(NCORES):
        m = core_of == c
        gidx = np.zeros(NPAD, np.int16)  # dummy -> row 0
        slots = np.full(NPAD, 255, np.float32)  # dummy -> no slot match
        gidx[stream_pos[m]] = trow_s[m].astype(np.int16)
        slots[stream_pos[m]] = slot_s[m].astype(np.float32)
        # wrap gidx for dma_gather: block b, idx j -> [j%16, b*256 + j//16]
        blk = np.zeros(B * GBLK * P, np.int16)
        blk[:NPAD] = gidx
        blk = blk.reshape(B, GBLK * P // 16, 16).transpose(0, 2, 1)  # [B,16,256]
        blk = np.tile(blk, (1, 8, 1))  # [B,128,256]
        gidx_in = blk.transpose(1, 0, 2).reshape(P, -1).copy()  # [128, B*256]
        slots_in = slots.reshape(T_total, P).T.copy()  # [128, T_total]
        per_core.append(dict(gidx=gidx_in, slots=slots_in))
    meta = dict(tile_window=tile_window.tolist(), T_total=T_total, B=B)
    return per_core, meta


# ---------------------------------------------------------------- builder


def build_kernel(cfg, meta, debug_phases=4, comm_mode="a2a"):
    SLICE, SLICE_PAD, TBL, DSLICE_PAD, WINDOWS = (
        cfg["SLICE"],
        cfg["SLICE_PAD"],
        cfg["TBL"],
        cfg["DSLICE_PAD"],
        cfg["WINDOWS"],
    )
    T_total, B, tile_window = meta["T_total"], meta["B"], meta["tile_window"]
    PTOT = NCORES * DSLICE_PAD
    f32 = mybir.dt.float32
    bf16 = mybir.dt.bfloat16
    SGRP = 8          # one-hot compare batch (tiles per DVE op)
    WQ = 4            # windows per PSUM bank (quad-window flush)
    assert WINDOWS % WQ == 0

    nc = bacc.Bacc(num_swdge_queues=4)

    def param(name, shape, dt=f32):
        return nc.declare_dram_parameter(name, list(shape), dt, isOutput=False)

    embp = [param(n, [P, SLICE_PAD], bf16) for n in ("embA", "embB", "embC")]
    w1 = [param(f"w1_{i}", [P, P], bf16) for i in range(4)]  # rel, hh0, hh1, loop
    b1 = [param(f"b1_{i}", [P, 1]) for i in range(4)]
    # W2/b2 pre-scaled by hop_coef on the host
    w2s = [param(f"w2s_{i}", [P, P], bf16) for i in range(4)]
    b2s = [param(f"b2s_{i}", [P, P]) for i in range(4)]  # row-broadcast
    iota_p = param("iota", [P, P])
    gidx_p = param("gidx", [P, B * 256], mybir.dt.int16)
    slots_p = param("slots", [P, T_total])
    tok_p = param("tok", [1, 1])
    out_ext = nc.declare_dram_parameter("out", [SLICE, C], f32, isOutput=True)
    tok_out = nc.declare_dram_parameter("tok_out", [1, 1], f32, isOutput=True)

    h_dram = nc.dram_tensor("h_tbl", [TBL, C], bf16)
    partial = nc.dram_tensor("partial", [PTOT, C], bf16)
    rs_out = nc.dram_tensor("rs_out", [DSLICE_PAD, C], bf16)
    a2a_out = nc.dram_tensor("a2a_out", [PTOT, C], bf16)

    def batched_rows_ap(handle, r0, nsub):
        # [p, s, ch] view of rows [r0, r0 + nsub*128) of a [rows, C] tensor
        return bass.AP(handle, r0 * C, [[C, P], [P * C, nsub], [1, C]])

    with tile.TileContext(nc) as tc:
        with (
            tc.tile_pool(name="resident", bufs=1) as rpool,
            tc.tile_pool(name="hid", bufs=3) as hpool,
            tc.tile_pool(name="hstage", bufs=4) as opool,
            tc.tile_pool(name="gather", bufs=4) as gpool,
            tc.tile_pool(name="onehot", bufs=3) as spool,
            tc.tile_pool(name="flush", bufs=4) as fpool,
            tc.tile_pool(name="fin", bufs=2) as finpool,
            tc.tile_pool(name="psA", bufs=2, space="PSUM") as psA,
            tc.tile_pool(name="psB", bufs=3, space="PSUM") as psB,
            tc.tile_pool(name="win", bufs=2, space="PSUM") as wpool,
        ):
            # ---- phase 0: resident loads
            w1_sb = [rpool.tile([P, P], bf16, tag=f"w1_{i}", name=f"w1sb{i}") for i in range(4)]
            b1_sb = [rpool.tile([P, 1], f32, tag=f"b1_{i}", name=f"b1sb{i}") for i in range(4)]
            w2s_sb = [rpool.tile([P, P], bf16, tag=f"w2s_{i}", name=f"w2ssb{i}") for i in range(4)]
            b2s_sb = [rpool.tile([P, P], f32, tag=f"b2s_{i}", name=f"b2ssb{i}") for i in range(4)]
            emb_sb = [rpool.tile([P, SLICE_PAD], bf16, tag=f"emb_{i}", name=f"embsb{i}") for i in range(3)]
            iota_sb = rpool.tile([P, P], f32, tag="iota")
            gidx_sb = rpool.tile([P, B * 256], mybir.dt.int16, tag="gidx")
            slots_sb = rpool.tile([P, T_total], f32, tag="slots")

            for i in range(4):
                nc.sync.dma_start(out=w1_sb[i][:], in_=w1[i][:, :])
                nc.sync.dma_start(out=b1_sb[i][:], in_=b1[i][:, :])
                nc.sync.dma_start(out=w2s_sb[i][:], in_=w2s[i][:, :])
                nc.sync.dma_start(out=b2s_sb[i][:], in_=b2s[i][:, :])
            for i in range(3):
                nc.sync.dma_start(out=emb_sb[i][:], in_=embp[i][:, :])
            nc.sync.dma_start(out=iota_sb[:], in_=iota_p[:, :])
            nc.sync.dma_start(out=gidx_sb[:], in_=gidx_p[:, :])
            nc.sync.dma_start(out=slots_sb[:], in_=slots_p[:, :])

            # ---- phase 1: MLP tables (hops 1..3 from embA/B/C; self from embA)
            # self table stays resident in SBUF (read directly by the finale)
            n_sub_tot = SLICE_PAD // P
            self_sb = rpool.tile([P, n_sub_tot, P], f32, tag="selft", name="selft")
            CH = 512
            n_chunks = math.ceil(SLICE_PAD / CH)
            for t in range(4):
                src = emb_sb[min(t, 2)] if t < 3 else emb_sb[0]
                row0 = t * SLICE_PAD if t < 3 else 0
                for j in range(n_chunks):
                    c0 = j * CH
                    cw = min(CH, SLICE_PAD - c0)
                    nsub = cw // P
                    ps1 = psA.tile([P, CH], f32, tag="l1", name="ps1")
                    nc.tensor.matmul(
                        ps1[:, :cw], w1_sb[t][:], src[:, c0 : c0 + cw],
                        start=True, stop=True,
                    )
                    u_sb = hpool.tile([P, CH], bf16, tag="u", name="u")
                    nc.scalar.activation(
                        u_sb[:, :cw], ps1[:, :cw],
                        mybir.ActivationFunctionType.Relu, bias=b1_sb[t][:],
                    )
                    st = None
                    if t < 3:
                        st = opool.tile([P, 4, P], bf16, tag="hst", name="hst")
                    for sub in range(nsub):
                        ps2 = psB.tile([P, P], f32, tag="l2", name="ps2")
                        nc.tensor.matmul(
                            ps2[:], u_sb[:, sub * P : (sub + 1) * P], w2s_sb[t][:],
                            start=True, stop=True,
                        )
                        dst_ap = (
                            st[:, sub, :] if t < 3
                            else self_sb[:, c0 // P + sub, :]
                        )
                        nc.vector.tensor_tensor(
                            out=dst_ap, in0=ps2[:], in1=b2s_sb[t][:],
                            op=mybir.AluOpType.add,
                        )
                    if t < 3:
                        nc.sync.dma_start(
                            out=batched_rows_ap(h_dram, row0 + c0, nsub),
                            in_=st[:, :nsub, :],
                        )

            tc.strict_bb_all_engine_barrier()

            def debug_out(srct, dt=f32):
                for j in range(math.ceil(SLICE / P)):
                    r0 = j * P
                    rw = min(P, SLICE - r0)
                    d_sb = finpool.tile([P, C], dt, tag="dbg", name="dbg")
                    nc.sync.dma_start(out=d_sb[:], in_=srct[r0 : r0 + P, :])
                    if dt != f32:
                        d32 = finpool.tile([P, C], f32, tag="dbg32", name="dbg32")
                        nc.scalar.copy(d32[:], d_sb[:])
                        d_sb = d32
                    nc.sync.dma_start(out=out_ext[r0 : r0 + rw, :], in_=d_sb[:rw, :])
                t_sb = finpool.tile([1, 1], f32, tag="tok", name="tok2")
                nc.sync.dma_start(out=t_sb[:], in_=tok_p[:, :])
                nc.sync.dma_start(out=tok_out[:, :], in_=t_sb[:])

            run_p2 = debug_phases >= 2
            run_p3 = debug_phases >= 3
            run_p4 = debug_phases >= 4
            if not run_p2:
                debug_out(h_dram, bf16)

            # ---- phase 2: gather + one-hot matmul accumulate, quad-window PSUM
            ps_q = None
            s_blk = None
            for b in range(B if run_p2 else 0):
                gbuf = gpool.tile([P, GBLK, C], bf16, tag="g", name="g")
                nc.gpsimd.dma_gather(
                    gbuf[:], h_dram.ap(),
                    gidx_sb[:, b * 256 : (b + 1) * 256],
                    GBLK * P, GBLK * P, C, single_packet=False,
                    queue_num=b % 4,
                )
                for tj in range(GBLK):
                    t_idx = b * GBLK + tj
                    if t_idx >= T_total:
                        break
                    if t_idx % SGRP == 0:
                        n_in_grp = min(SGRP, T_total - t_idx)
                        s_blk = spool.tile([P, SGRP, P], bf16, tag="s", name="sblk")
                        nc.vector.tensor_tensor(
                            out=s_blk[:, :n_in_grp, :],
                            in0=slots_sb[:, t_idx : t_idx + n_in_grp]
                            .unsqueeze(2).broadcast_to([P, n_in_grp, P]),
                            in1=iota_sb[:].unsqueeze(1).broadcast_to([P, n_in_grp, P]),
                            op=mybir.AluOpType.is_equal,
                        )
                    w = tile_window[t_idx]
                    q, wi = w // WQ, w % WQ
                    first = t_idx == 0 or tile_window[t_idx - 1] != w
                    last = t_idx == T_total - 1 or tile_window[t_idx + 1] != w
                    q_first = first and (wi == 0 or tile_window[t_idx - 1] // WQ != q)
                    q_last = last and (
                        t_idx == T_total - 1 or tile_window[t_idx + 1] // WQ != q
                    )
                    if q_first:
                        ps_q = wpool.tile([P, WQ * P], f32, tag="w", name="psq")
                    nc.tensor.matmul(
                        ps_q[:, wi * P : (wi + 1) * P],
                        s_blk[:, t_idx % SGRP, :],
                        gbuf[:, tj, :],
                        start=first, stop=last,
                    )
                    if q_last:
                        f_sb = fpool.tile([P, WQ, P], bf16, tag="f", name="fsb")
                        nc.scalar.copy(f_sb[:], ps_q[:].rearrange("p (s c) -> p s c", s=WQ))
                        nc.sync.dma_start(
                            out=batched_rows_ap(partial, q * WQ * P, WQ),
                            in_=f_sb[:],
                        )

            tc.strict_bb_all_engine_barrier()

            if run_p2 and not run_p3:
                debug_out(partial, bf16)

            # ---- phase 3+4: combine partials across cores, add self, write out
            n_fin = math.ceil(SLICE / P)
            if run_p3 and comm_mode == "a2a":
                nc.gpsimd.collective_compute(
                    "AllToAll", mybir.AluOpType.bypass,
                    replica_groups=[list(range(NCORES))],
                    ins=[partial.ap()], outs=[a2a_out.ap()],
                )
                tc.strict_bb_all_engine_barrier()
                if not run_p4:
                    debug_out(a2a_out, bf16)
                GF = 8
                j0 = 0
                while j0 < (n_fin if run_p4 else 0):
                    g = min(GF, n_fin - j0)
                    r0 = j0 * P
                    r_sb = finpool.tile([P, NCORES, GF, P], bf16, tag="fa", name="fa")
                    # one bulk linear read per source core's copy of this
                    # g-row-tile group
                    for s in range(NCORES):
                        nc.sync.dma_start(
                            out=r_sb[:, s, :g, :],
                            in_=bass.AP(
                                a2a_out,
                                (s * DSLICE_PAD + r0) * C,
                                [[C, P], [P * C, g], [1, C]],
                            ),
                        )
                    nc.vector.tensor_tensor(
                        out=r_sb[:, 0:4, :g, :], in0=r_sb[:, 0:4, :g, :],
                        in1=r_sb[:, 4:8, :g, :], op=mybir.AluOpType.add,
                    )
                    nc.vector.tensor_tensor(
                        out=r_sb[:, 0:2, :g, :], in0=r_sb[:, 0:2, :g, :],
                        in1=r_sb[:, 2:4, :g, :], op=mybir.AluOpType.add,
                    )
                    h32 = finpool.tile([P, GF, P], f32, tag="fh", name="fh")
                    nc.vector.tensor_tensor(
                        out=h32[:, :g, :], in0=r_sb[:, 0, :g, :],
                        in1=r_sb[:, 1, :g, :], op=mybir.AluOpType.add,
                    )
                    o_sb = finpool.tile([P, GF, P], f32, tag="fo", name="fo")
                    nc.vector.tensor_tensor(
                        out=o_sb[:, :g, :], in0=h32[:, :g, :],
                        in1=self_sb[:, j0 : j0 + g, :], op=mybir.AluOpType.add,
                    )
                    if (j0 + g) * P <= SLICE:
                        nc.sync.dma_start(
                            out=batched_rows_ap(out_ext, r0, g),
                            in_=o_sb[:, :g, :],
                        )
                    else:
                        for jj in range(g):
                            rr = (j0 + jj) * P
                            rw = min(P, SLICE - rr)
                            if rw <= 0:
                                break
                            nc.sync.dma_start(
                                out=out_ext[rr : rr + rw, :],
                                in_=o_sb[:rw, jj, :],
                            )
                    j0 += g
            elif run_p3:
                nc.gpsimd.collective_compute(
                    "ReduceScatter", mybir.AluOpType.add,
                    replica_groups=[list(range(NCORES))],
                    ins=[partial.ap()], outs=[rs_out.ap()],
                )
                tc.strict_bb_all_engine_barrier()
                if not run_p4:
                    debug_out(rs_out, bf16)
                for j in range(n_fin if run_p4 else 0):
                    r0 = j * P
                    rw = min(P, SLICE - r0)
                    a_sb = finpool.tile([P, C], bf16, tag="fa2", name="fa2")
                    nc.sync.dma_start(out=a_sb[:], in_=rs_out[r0 : r0 + P, :])
                    a32 = finpool.tile([P, C], f32, tag="fh2", name="fh2")
                    nc.scalar.copy(a32[:], a_sb[:])
                    o_sb = finpool.tile([P, C], f32, tag="fo2", name="fo2")
                    nc.vector.tensor_tensor(
                        out=o_sb[:], in0=a32[:], in1=self_sb[:, j, :],
                        op=mybir.AluOpType.add,
                    )
                    nc.sync.dma_start(out=out_ext[r0 : r0 + rw, :], in_=o_sb[:rw, :])

            if run_p4:
                t_sb = finpool.tile([1, 1], f32, tag="tok", name="tokf")
                nc.sync.dma_start(out=t_sb[:], in_=tok_p[:, :])
                nc.sync.dma_start(out=tok_out[:, :], in_=t_sb[:])

    nc.compile()
    return nc


# ---------------------------------------------------------------- entry


def make_in_maps(cfg, inputs):
    """Full problem inputs -> per-core in_maps (+ meta)."""
    N, SLICE, SLICE_PAD = cfg["N"], cfg["SLICE"], cfg["SLICE_PAD"]
    ne = np.asarray(inputs["node_embeddings"], np.float32)
    t = int(inputs["t"])
    assert t == 2 and ne.shape[0] == 3
    ei = np.asarray(inputs["edge_index"])
    ew = np.asarray(inputs["edge_weights"])
    per_core_edges, meta = prep_edges(cfg, ei[0], ei[1], ew)

    # per-hop source embedding layers: hop1 -> ne[t], hop2 -> ne[t-1], hop3 -> ne[t-2]
    layers = [ne[2], ne[1], ne[0]]
    hop_coef = np.asarray(inputs["hop_coef"], np.float32)
    coef4 = np.concatenate([hop_coef, [1.0]]).astype(np.float32)
    iota_in = np.broadcast_to(
        np.arange(P, dtype=np.float32)[None, :], (P, P)
    ).copy()

    w_names = [
        ("rel_W1", "rel_b1", "rel_W2", "rel_b2"),
        None,  # hh index 0
        None,  # hh index 1
        ("loop_W1", "loop_b1", "loop_W2", "loop_b2"),
    ]

    def wset(i):
        if i in (1, 2):
            W1 = np.asarray(inputs["hh_W1"][i - 1], np.float32)
            bb1 = np.asarray(inputs["hh_b1"][i - 1], np.float32)
            W2 = np.asarray(inputs["hh_W2"][i - 1], np.float32)
            bb2 = np.asarray(inputs["hh_b2"][i - 1], np.float32)
        else:
            n1, n2, n3, n4 = w_names[i]
            W1 = np.asarray(inputs[n1], np.float32)
            bb1 = np.asarray(inputs[n2], np.float32)
            W2 = np.asarray(inputs[n3], np.float32)
            bb2 = np.asarray(inputs[n4], np.float32)
        # pre-scale second-layer weights/bias by this hop's coefficient
        return (
            np.ascontiguousarray(W1).astype(ml_dtypes.bfloat16),
            np.ascontiguousarray(bb1[:, None]),
            np.ascontiguousarray(W2 * coef4[i]).astype(ml_dtypes.bfloat16),
            np.broadcast_to((bb2 * coef4[i])[None, :], (P, P)).copy(),
        )

    wsets = [wset(i) for i in range(4)]

    in_maps = []
    for c in range(NCORES):
        m = {}
        for li, name in enumerate(("embA", "embB", "embC")):
            sl = layers[li][c * SLICE : (c + 1) * SLICE]
            pad = np.zeros((P, SLICE_PAD), ml_dtypes.bfloat16)
            pad[:, : sl.shape[0]] = sl.T.astype(ml_dtypes.bfloat16)
            m[name] = pad
        for i in range(4):
            W1, bb1, W2s, bb2s = wsets[i]
            m[f"w1_{i}"] = W1
            m[f"b1_{i}"] = bb1
            m[f"w2s_{i}"] = W2s
            m[f"b2s_{i}"] = bb2s
        m["iota"] = iota_in
        m["gidx"] = per_core_edges[c]["gidx"]
        m["slots"] = per_core_edges[c]["slots"]
        m["tok"] = np.zeros((1, 1), np.float32)
        in_maps.append(m)
    return in_maps, meta


def kernel(**inputs):
    ei = np.asarray(inputs["edge_index"])
    ne = np.asarray(inputs["node_embeddings"])
    cfg = make_cfg(ne.shape[1], ei.shape[1])
    in_maps, meta = make_in_maps(cfg, inputs)
    nc = build_kernel(cfg, meta)
    res = run_bass_kernel_spmd(nc, in_maps, core_ids=list(range(NCORES)))
    out = np.concatenate([res.results[c]["out"] for c in range(NCORES)], axis=0)
    return out.astype(np.float32)



# revision 24
# speedup vs baseline: 7.5038x; 1.5153x over previous
"""DRew-GIN layer on 8 TRN2 NeuronCores.

Strategy (source-sharded, no table replication):
  - Nodes are sharded 8 ways. Core c computes the three coef-scaled hop MLP
    tables h'_k = hop_coef[k-1] * MLP_k(emb_src_k) for its node slice only,
    node-major bf16 in local DRAM (W2/b2 are pre-scaled by hop_coef on the
    host; MLP runs bf16 in / f32 PSUM).  The self-loop MLP table for the
    slice stays resident in SBUF.
  - Edges are partitioned by SOURCE core. Each core produces a PARTIAL
    aggregate over the full (padded) destination range: edges are sorted by
    destination window (128 dest rows); per 128-edge tile we dma_gather the
    bf16 source rows (256B each, spread over 4 SWDGE queues) from the local
    h' table, build a one-hot selection matrix S^T[e, d] = (slot[e] == d) on
    DVE, and matmul-accumulate S^T.T @ G into the window's PSUM tile (f32).
    Window flushes convert to bf16 into a partial table [8*DSLICE_PAD, 128].
  - One AllToAll (bypass) exchanges the bf16 partials; the finale reads the
    8 per-source copies of each 8-row-tile group with bulk linear DMAs,
    tree-adds them on DVE (bf16 -> f32), adds the SBUF-resident self term,
    and writes the output slice.

bf16 halves gather/flush/collective bytes; accumulation stays f32 in PSUM
and the final add is f32 (rel err ~4e-3 vs the f32 reference).
"""

import math
import sys

sys.path.insert(0, "/opt/trn_rl_repo")

import ml_dtypes
import numpy as np

import concourse.bacc as bacc
import concourse.bass as bass
import concourse.tile as tile
from concourse import mybir
from concourse.bass_utils import run_bass_kernel_spmd

NCORES = 8
C = 128
P = 128
GBLK = 32  # gather block = 32 tiles = 4096 indices


def make_cfg(n_nodes, n_edges):
    assert n_nodes % NCORES == 0
    slice_ = n_nodes // NCORES
    slice_pad = ((slice_ + P - 1) // P) * P
    wps = slice_pad // P  # windows per dest slice
    cfg = dict(
        N=n_nodes,
        E=n_edges,
        SLICE=slice_,
        SLICE_PAD=slice_pad,
        TBL=3 * slice_pad,  # h' table rows per core
        DSLICE_PAD=slice_pad,
        WINDOWS=NCORES * wps,
        WPS=wps,
    )
    return cfg


# ---------------------------------------------------------------- host prep


def prep_edges(cfg, row, col, ew):
    """Returns (per_core {gidx,slots}, meta {tile_window, T_total, B})."""
    N, SLICE, SLICE_PAD, DSLICE_PAD, WINDOWS = (
        cfg["N"],
        cfg["SLICE"],
        cfg["SLICE_PAD"],
        cfg["DSLICE_PAD"],
        cfg["WINDOWS"],
    )
    row = row.astype(np.int64)
    col = col.astype(np.int64)
    ew = ew.astype(np.int64)
    s = col // SLICE
    local = col - s * SLICE
    trow = (ew - 1) * SLICE_PAD + local
    assert trow.max() < 3 * SLICE_PAD <= 32767
    dp = (row // SLICE) * DSLICE_PAD + (row % SLICE)
    w = dp // P
    slot = dp % P

    key = s * WINDOWS + w
    order = np.argsort(key, kind="stable")
    key_s = key[order]
    counts = np.bincount(key_s, minlength=NCORES * WINDOWS).reshape(NCORES, WINDOWS)
    tw = np.maximum(1, (counts.max(axis=0) + P - 1) // P)  # [WINDOWS]
    T_total = int(tw.sum())
    B = (T_total + GBLK - 1) // GBLK
    tile_window = np.repeat(np.arange(WINDOWS), tw)  # [T_total]
    win_tile_off = np.concatenate([[0], np.cumsum(tw)])[:-1]  # [WINDOWS]

    # position of each (sorted) edge inside its (core, window) group
    group_starts = np.concatenate([[0], np.cumsum(counts.reshape(-1))])[:-1]
    pos_in_group = np.arange(len(key_s)) - group_starts[key_s]
    # destination slot index in the padded per-core stream
    core_of = key_s // WINDOWS
    win_of = key_s % WINDOWS
    stream_pos = win_tile_off[win_of] * P + pos_in_group

    NPAD = T_total * P
    per_core = []
    trow_s = trow[order]
    slot_s = slot[order]
    for c in range(NCORES):
        m = core_of == c
        gidx = np.zeros(NPAD, np.int16)  # dummy -> row 0
        slots = np.full(NPAD, 255, np.float32)  # dummy -> no slot match
        gidx[stream_pos[m]] = trow_s[m].astype(np.int16)
        slots[stream_pos[m]] = slot_s[m].astype(np.float32)
        # wrap gidx for dma_gather: block b, idx j -> [j%16, b*256 + j//16]
        blk = np.zeros(B * GBLK * P, np.int16)
        blk[:NPAD] = gidx
        blk = blk.reshape(B, GBLK * P // 16, 16).transpose(0, 2, 1)  # [B,16,256]
        blk = np.tile(blk, (1, 8, 1))  # [B,128,256]
        gidx_in = blk.transpose(1, 0, 2).reshape(P, -1).copy()  # [128, B*256]
        slots_in = slots.reshape(T_total, P).T.copy()  # [128, T_total]
        per_core.append(dict(gidx=gidx_in, slots=slots_in))
    meta = dict(tile_window=tile_window.tolist(), T_total=T_total, B=B)
    return per_core, meta


# ---------------------------------------------------------------- builder


def build_kernel(cfg, meta, debug_phases=4, comm_mode="a2a"):
    SLICE, SLICE_PAD, TBL, DSLICE_PAD, WINDOWS = (
        cfg["SLICE"],
        cfg["SLICE_PAD"],
        cfg["TBL"],
        cfg["DSLICE_PAD"],
        cfg["WINDOWS"],
    )
    T_total, B, tile_window = meta["T_total"], meta["B"], meta["tile_window"]
    PTOT = NCORES * DSLICE_PAD
    f32 = mybir.dt.float32
    bf16 = mybir.dt.bfloat16
    SGRP = 8          # one-hot compare batch (tiles per DVE op)
    WQ = 4            # windows per PSUM bank (quad-window flush)
    assert WINDOWS % WQ == 0

    nc = bacc.Bacc(num_swdge_queues=4)

    def param(name, shape, dt=f32):
        return nc.declare_dram_parameter(name, list(shape), dt, isOutput=False)

    embp = [param(n, [P, SLICE_PAD], bf16) for n in ("embA", "embB", "embC")]
    w1 = [param(f"w1_{i}", [P, P], bf16) for i in range(4)]  # rel, hh0, hh1, loop
    b1 = [param(f"b1_{i}", [P, 1]) for i in range(4)]
    # W2/b2 pre-scaled by hop_coef on the host
    w2s = [param(f"w2s_{i}", [P, P], bf16) for i in range(4)]
    b2s = [param(f"b2s_{i}", [P, P]) for i in range(4)]  # row-broadcast
    iota_p = param("iota", [P, P])
    gidx_p = param("gidx", [P, B * 256], mybir.dt.int16)
    slots_p = param("slots", [P, T_total])
    tok_p = param("tok", [1, 1])
    out_ext = nc.declare_dram_parameter("out", [SLICE, C], f32, isOutput=True)
    tok_out = nc.declare_dram_parameter("tok_out", [1, 1], f32, isOutput=True)

    h_dram = nc.dram_tensor("h_tbl", [TBL, C], bf16)
    partial = nc.dram_tensor("partial", [PTOT, C], bf16)
    rs_out = nc.dram_tensor("rs_out", [DSLICE_PAD, C], bf16)
    a2a_out = nc.dram_tensor("a2a_out", [PTOT, C], bf16)

    def batched_rows_ap(handle, r0, nsub):
        # [p, s, ch] view of rows [r0, r0 + nsub*128) of a [rows, C] tensor
        return bass.AP(handle, r0 * C, [[C, P], [P * C, nsub], [1, C]])

    with tile.TileContext(nc) as tc:
        with (
            tc.tile_pool(name="resident", bufs=1) as rpool,
            tc.tile_pool(name="hid", bufs=3) as hpool,
            tc.tile_pool(name="hstage", bufs=4) as opool,
            tc.tile_pool(name="gather", bufs=4) as gpool,
            tc.tile_pool(name="onehot", bufs=3) as spool,
            tc.tile_pool(name="flush", bufs=4) as fpool,
            tc.tile_pool(name="fin", bufs=2) as finpool,
            tc.tile_pool(name="psA", bufs=2, space="PSUM") as psA,
            tc.tile_pool(name="psB", bufs=3, space="PSUM") as psB,
            tc.tile_pool(name="win", bufs=2, space="PSUM") as wpool,
        ):
            # ---- phase 0: resident loads
            w1_sb = [rpool.tile([P, P], bf16, tag=f"w1_{i}", name=f"w1sb{i}") for i in range(4)]
            b1_sb = [rpool.tile([P, 1], f32, tag=f"b1_{i}", name=f"b1sb{i}") for i in range(4)]
            w2s_sb = [rpool.tile([P, P], bf16, tag=f"w2s_{i}", name=f"w2ssb{i}") for i in range(4)]
            b2s_sb = [rpool.tile([P, P], f32, tag=f"b2s_{i}", name=f"b2ssb{i}") for i in range(4)]
            emb_sb = [rpool.tile([P, SLICE_PAD], bf16, tag=f"emb_{i}", name=f"embsb{i}") for i in range(3)]
            iota_sb = rpool.tile([P, P], f32, tag="iota")
            gidx_sb = rpool.tile([P, B * 256], mybir.dt.int16, tag="gidx")
            slots_sb = rpool.tile([P, T_total], f32, tag="slots")

            for i in range(4):
                nc.sync.dma_start(out=w1_sb[i][:], in_=w1[i][:, :])
                nc.sync.dma_start(out=b1_sb[i][:], in_=b1[i][:, :])
                nc.sync.dma_start(out=w2s_sb[i][:], in_=w2s[i][:, :])
                nc.sync.dma_start(out=b2s_sb[i][:], in_=b2s[i][:, :])
            for i in range(3):
                nc.sync.dma_start(out=emb_sb[i][:], in_=embp[i][:, :])
            nc.sync.dma_start(out=iota_sb[:], in_=iota_p[:, :])
            nc.sync.dma_start(out=gidx_sb[:], in_=gidx_p[:, :])
            nc.sync.dma_start(out=slots_sb[:], in_=slots_p[:, :])

            # ---- phase 1: MLP tables (hops 1..3 from embA/B/C; self from embA)
            # self table stays resident in SBUF (read directly by the finale)
            n_sub_tot = SLICE_PAD // P
            self_sb = rpool.tile([P, n_sub_tot, P], f32, tag="selft", name="selft")
            CH = 512
            n_chunks = math.ceil(SLICE_PAD / CH)
            for t in range(4):
                src = emb_sb[min(t, 2)] if t < 3 else emb_sb[0]
                row0 = t * SLICE_PAD if t < 3 else 0
                for j in range(n_chunks):
                    c0 = j * CH
                    cw = min(CH, SLICE_PAD - c0)
                    nsub = cw // P
                    ps1 = psA.tile([P, CH], f32, tag="l1", name="ps1")
                    nc.tensor.matmul(
                        ps1[:, :cw], w1_sb[t][:], src[:, c0 : c0 + cw],
                        start=True, stop=True,
                    )
                    u_sb = hpool.tile([P, CH], bf16, tag="u", name="u")
                    nc.scalar.activation(
                        u_sb[:, :cw], ps1[:, :cw],
                        mybir.ActivationFunctionType.Relu, bias=b1_sb[t][:],
                    )
                    st = None
                    if t < 3:
                        st = opool.tile([P, 4, P], bf16, tag="hst", name="hst")
                    for sub in range(nsub):
                        ps2 = psB.tile([P, P], f32, tag="l2", name="ps2")
                        nc.tensor.matmul(
                            ps2[:], u_sb[:, sub * P : (sub + 1) * P], w2s_sb[t][:],
                            start=True, stop=True,
                        )
                        dst_ap = (
                            st[:, sub, :] if t < 3
                            else self_sb[:, c0 // P + sub, :]
                        )
                        nc.vector.tensor_tensor(
                            out=dst_ap, in0=ps2[:], in1=b2s_sb[t][:],
                            op=mybir.AluOpType.add,
                        )
                    if t < 3:
                        nc.sync.dma_start(
                            out=batched_rows_ap(h_dram, row0 + c0, nsub),
                            in_=st[:, :nsub, :],
                        )

            tc.strict_bb_all_engine_barrier()

            def debug_out(srct, dt=f32):
                for j in range(math.ceil(SLICE / P)):
                    r0 = j * P
                    rw = min(P, SLICE - r0)
                    d_sb = finpool.tile([P, C], dt, tag="dbg", name="dbg")
                    nc.sync.dma_start(out=d_sb[:], in_=srct[r0 : r0 + P, :])
                    if dt != f32:
                        d32 = finpool.tile([P, C], f32, tag="dbg32", name="dbg32")
                        nc.scalar.copy(d32[:], d_sb[:])
                        d_sb = d32
                    nc.sync.dma_start(out=out_ext[r0 : r0 + rw, :], in_=d_sb[:rw, :])
                t_sb = finpool.tile([1, 1], f32, tag="tok", name="tok2")
                nc.sync.dma_start(out=t_sb[:], in_=tok_p[:, :])
                nc.sync.dma_start(out=tok_out[:, :], in_=t_sb[:])

            run_p2 = debug_phases >= 2
            run_p3 = debug_phases >= 3
            run_p4 = debug_phases >= 4
            if not run_p2:
                debug_out(h_dram, bf16)

            # ---- phase 2: gather + one-hot matmul accumulate, quad-window PSUM
            ps_q = None
            s_blk = None
            for b in range(B if run_p2 else 0):
                gbuf = gpool.tile([P, GBLK, C], bf16, tag="g", name="g")
                nc.gpsimd.dma_gather(
                    gbuf[:], h_dram.ap(),
                    gidx_sb[:, b * 256 : (b + 1) * 256],
                    GBLK * P, GBLK * P, C, single_packet=False,
                    queue_num=b % 4,
                )
                for tj in range(GBLK):
                    t_idx = b * GBLK + tj
                    if t_idx >= T_total:
                        break
                    if t_idx % SGRP == 0:
                        n_in_grp = min(SGRP, T_total - t_idx)
                        s_blk = spool.tile([P, SGRP, P], bf16, tag="s", name="sblk")
                        nc.vector.tensor_tensor(
                            out=s_blk[:, :n_in_grp, :],
                            in0=slots_sb[:, t_idx : t_idx + n_in_grp]
                            .unsqueeze(2).broadcast_to([P, n_in_grp, P]),
                            in1=iota_sb[:].unsqueeze(1).broadcast_to([P, n_in_grp, P]),
                            op=mybir.AluOpType.is_equal,
                        )
                    w = tile_window[t_idx]
                    q, wi = w // WQ, w % WQ
                    first = t_idx == 0 or tile_window[t_idx - 1] != w
                    last = t_idx == T_total - 1 or tile_window[t_idx + 1] != w
                    q_first = first and (wi == 0 or tile_window[t_idx - 1] // WQ != q)
                    q_last = last and (
                        t_idx == T_total - 1 or tile_window[t_idx + 1] // WQ != q
                    )
                    if q_first:
                        ps_q = wpool.tile([P, WQ * P], f32, tag="w", name="psq")
                    nc.tensor.matmul(
                        ps_q[:, wi * P : (wi + 1) * P],
                        s_blk[:, t_idx % SGRP, :],
                        gbuf[:, tj, :],
                        start=first, stop=last,
                    )
                    if q_last:
                        f_sb = fpool.tile([P, WQ, P], bf16, tag="f", name="fsb")
                        nc.scalar.copy(f_sb[:], ps_q[:].rearrange("p (s c) -> p s c", s=WQ))
                        nc.sync.dma_start(
                            out=batched_rows_ap(partial, q * WQ * P, WQ),
                            in_=f_sb[:],
                        )

            tc.strict_bb_all_engine_barrier()

            if run_p2 and not run_p3:
                debug_out(partial, bf16)

            # ---- phase 3+4: combine partials across cores, add self, write out
            n_fin = math.ceil(SLICE / P)
            if run_p3 and comm_mode == "a2a":
                nc.gpsimd.collective_compute(
                    "AllToAll", mybir.AluOpType.bypass,
                    replica_groups=[list(range(NCORES))],
                    ins=[partial.ap()], outs=[a2a_out.ap()],
                )
                tc.strict_bb_all_engine_barrier()
                if not run_p4:
                    debug_out(a2a_out, bf16)
                GF = 8
                j0 = 0
                while j0 < (n_fin if run_p4 else 0):
                    g = min(GF, n_fin - j0)
                    r0 = j0 * P
                    r_sb = finpool.tile([P, NCORES, GF, P], bf16, tag="fa", name="fa")
                    # one bulk linear read per source core's copy of this
                    # g-row-tile group
                    for s in range(NCORES):
                        nc.sync.dma_start(
                            out=r_sb[:, s, :g, :],
                            in_=bass.AP(
                                a2a_out,
                                (s * DSLICE_PAD + r0) * C,
                                [[C, P], [P * C, g], [1, C]],
                            ),
                        )
                    nc.vector.tensor_tensor(
                        out=r_sb[:, 0:4, :g, :], in0=r_sb[:, 0:4, :g, :],
                        in1=r_sb[:, 4:8, :g, :], op=mybir.AluOpType.add,
                    )
                    nc.vector.tensor_tensor(
                        out=r_sb[:, 0:2, :g, :], in0=r_sb[:, 0:2, :g, :],
                        in1=r_sb[:, 2:4, :g, :], op=mybir.AluOpType.add,
                    )
                    h32 = finpool.tile([P, GF, P], f32, tag="fh", name="fh")
                    nc.vector.tensor_tensor(
                        out=h32[:, :g, :], in0=r_sb[:, 0, :g, :],
                        in1=r_sb[:, 1, :g, :], op=mybir.AluOpType.add,
                    )
                    o_sb = finpool.tile([P, GF, P], f32, tag="fo", name="fo")
                    nc.vector.tensor_tensor(
                        out=o_sb[:, :g, :], in0=h32[:, :g, :],
                        in1=self_sb[:, j0 : j0 + g, :], op=mybir.AluOpType.add,
                    )
                    if (j0 + g) * P <= SLICE:
                        nc.sync.dma_start(
                            out=batched_rows_ap(out_ext, r0, g),
                            in_=o_sb[:, :g, :],
                        )
                    else:
                        for jj in range(g):
                            rr = (j0 + jj) * P
                            rw = min(P, SLICE - rr)
                            if rw <= 0:
                                break
                            nc.sync.dma_start(
                                out=out_ext[rr : rr + rw, :],
                                in_=o_sb[:rw, jj, :],
                            )
                    j0 += g
            elif run_p3:
                nc.gpsimd.collective_compute(
                    "ReduceScatter", mybir.AluOpType.add,
                    replica_groups=[list(range(NCORES))],
                    ins=[partial.ap()], outs=[rs_out.ap()],
                )
                tc.strict_bb_all_engine_barrier()
                if not run_p4:
                    debug_out(rs_out, bf16)
                for j in range(n_fin if run_p4 else 0):
                    r0 = j * P
                    rw = min(P, SLICE - r0)
                    a_sb = finpool.tile([P, C], bf16, tag="fa2", name="fa2")
                    nc.sync.dma_start(out=a_sb[:], in_=rs_out[r0 : r0 + P, :])
                    a32 = finpool.tile([P, C], f32, tag="fh2", name="fh2")
                    nc.scalar.copy(a32[:], a_sb[:])
                    o_sb = finpool.tile([P, C], f32, tag="fo2", name="fo2")
                    nc.vector.tensor_tensor(
                        out=o_sb[:], in0=a32[:], in1=self_sb[:, j, :],
                        op=mybir.AluOpType.add,
                    )
                    nc.sync.dma_start(out=out_ext[r0 : r0 + rw, :], in_=o_sb[:rw, :])

            if run_p4:
                t_sb = finpool.tile([1, 1], f32, tag="tok", name="tokf")
                nc.sync.dma_start(out=t_sb[:], in_=tok_p[:, :])
                nc.sync.dma_start(out=tok_out[:, :], in_=t_sb[:])

    nc.compile()
    return nc


# ---------------------------------------------------------------- entry


def make_in_maps(cfg, inputs):
    """Full problem inputs -> per-core in_maps (+ meta)."""
    N, SLICE, SLICE_PAD = cfg["N"], cfg["SLICE"], cfg["SLICE_PAD"]
    ne = np.asarray(inputs["node_embeddings"], np.float32)
    t = int(inputs["t"])
    assert t == 2 and ne.shape[0] == 3
    ei = np.asarray(inputs["edge_index"])
    ew = np.asarray(inputs["edge_weights"])
    per_core_edges, meta = prep_edges(cfg, ei[0], ei[1], ew)

    # per-hop source embedding layers: hop1 -> ne[t], hop2 -> ne[t-1], hop3 -> ne[t-2]
    layers = [ne[2], ne[1], ne[0]]
    hop_coef = np.asarray(inputs["hop_coef"], np.float32)
    coef4 = np.concatenate([hop_coef, [1.0]]).astype(np.float32)
    iota_in = np.broadcast_to(
        np.arange(P, dtype=np.float32)[None, :], (P, P)
    ).copy()

    w_names = [
        ("rel_W1", "rel_b1", "rel_W2", "rel_b2"),
        None,  # hh index 0
        None,  # hh index 1
        ("loop_W1", "loop_b1", "loop_W2", "loop_b2"),
    ]

    def wset(i):
        if i in (1, 2):
            W1 = np.asarray(inputs["hh_W1"][i - 1], np.float32)
            bb1 = np.asarray(inputs["hh_b1"][i - 1], np.float32)
            W2 = np.asarray(inputs["hh_W2"][i - 1], np.float32)
            bb2 = np.asarray(inputs["hh_b2"][i - 1], np.float32)
        else:
            n1, n2, n3, n4 = w_names[i]
            W1 = np.asarray(inputs[n1], np.float32)
            bb1 = np.asarray(inputs[n2], np.float32)
            W2 = np.asarray(inputs[n3], np.float32)
            bb2 = np.asarray(inputs[n4], np.float32)
        # pre-scale second-layer weights/bias by this hop's coefficient
        return (
            np.ascontiguousarray(W1).astype(ml_dtypes.bfloat16),
            np.ascontiguousarray(bb1[:, None]),
            np.ascontiguousarray(W2 * coef4[i]).astype(ml_dtypes.bfloat16),
            np.broadcast_to((bb2 * coef4[i])[None, :], (P, P)).copy(),
        )

    wsets = [wset(i) for i in range(4)]

    in_maps = []
    for c in range(NCORES):
        m = {}
        for li, name in enumerate(("embA", "embB", "embC")):
            sl = layers[li][c * SLICE : (c + 1) * SLICE]
            pad = np.zeros((P, SLICE_PAD), ml_dtypes.bfloat16)
            pad[:, : sl.shape[0]] = sl.T.astype(ml_dtypes.bfloat16)
            m[name] = pad
        for i in range(4):
            W1, bb1, W2s, bb2s = wsets[i]
            m[f"w1_{i}"] = W1
            m[f"b1_{i}"] = bb1
            m[f"w2s_{i}"] = W2s
            m[f"b2s_{i}"] = bb2s
        m["iota"] = iota_in
        m["gidx"] = per_core_edges[c]["gidx"]
        m["slots"] = per_core_edges[c]["slots"]
        m["tok"] = np.zeros((1, 1), np.float32)
        in_maps.append(m)
    return in_maps, meta


def kernel(**inputs):
    ei = np.asarray(inputs["edge_index"])
    ne = np.asarray(inputs["node_embeddings"])
    cfg = make_cfg(ne.shape[1], ei.shape[1])
    in_maps, meta = make_in_maps(cfg, inputs)
    nc = build_kernel(cfg, meta)
    res = run_bass_kernel_spmd(nc, in_maps, core_ids=list(range(NCORES)))
    out = np.concatenate([res.results[c]["out"] for c in range(NCORES)], axis=0)
    return out.astype(np.float32)

